# revision 9
# baseline (speedup 1.0000x reference)
"""Trainium2 Bass kernel for a 2-layer GAT block (gnn_message_passing).

Strategy (8 NeuronCores, dst-node sharding), v2:
  - fp16 node tables with rows of EXACTLY 512B (256 halves, head-major
    channel layout c*H+h) -> the per-edge row gather moves 512B instead of
    1536B (f32 row + separate a_dst row in v1).
  - Householder rotation trick: per head, rotate the 64-dim block so that
    row 0 of the rotation is a_src_h itself.  Then alpha_src per edge is
    just channels 0..3 (head-major c=0) of the gathered row -- no extra
    table columns.  The inverse rotations are folded into W2 / the final
    mean-over-heads matmul on the host.
  - a_dst per edge gathered from a core-local [npc, 128] fp16 table
    (256B elements, dst-local int16 indices, no AllGather).
  - Edges sorted by dst, sharded by dst range, grouped into 128-dst groups,
    split by src half so gather indices fit int16; groups processed in
    chunks of CG=2 with ONE dma_gather call per (chunk, stream).
  - Per 128-edge block: fp16 one-hot S (tensor_scalar is_equal), two fp16
    matmuls (segment-sum of scaled messages + softmax denominators) into a
    single PSUM tile.  exp(leaky_relu(as+ad)) batched per chunk on DVE/ACT;
    message scaling via one 4D-AP broadcast tensor_tensor per chunk.
"""

import numpy as np

import concourse.bass as bass
import concourse.bacc as bacc
import concourse.mybir as mybir
import concourse.tile as tile
from concourse.bass_utils import run_bass_kernel_spmd

N = 50000
E = 800000
IN_C = 128
OUT_C = 64
HEADS = 4
NEG_SLOPE = 0.2
N_CORES = 8

P = 128
CG = 2              # groups per gather chunk

FP32 = mybir.dt.float32
FP16 = mybir.dt.float16
I16 = mybir.dt.int16

# timing-triage flags
SKIP_EDGE = False
SKIP_AG = False
GATHER_ONLY = False
DEBUG = False


def _ceil_div(a, b):
    return (a + b - 1) // b


def _pack_idxs(flat):
    """flat[k] -> int16 [16, len/16] at (k%16, k//16), for dma_gather."""
    m = len(flat)
    assert m % 16 == 0
    arr = np.zeros((16, m // 16), np.int16)
    arr[np.arange(m) % 16, np.arange(m) // 16] = flat
    return arr


# ---------------------------------------------------------------------------
# Host-side preprocessing
# ---------------------------------------------------------------------------

def _preprocess(edge_index, n, n_cores):
    npc = n // n_cores
    G = _ceil_div(npc, P)
    split = (n + 1) // 2

    src = np.asarray(edge_index[0], dtype=np.int64)
    dst = np.asarray(edge_index[1], dtype=np.int64)
    loops = np.arange(n, dtype=np.int64)
    src = np.concatenate([src, loops]).astype(np.int32)
    dst = np.concatenate([dst, loops]).astype(np.int32)

    order = np.argsort(dst, kind="stable")
    src = src[order]
    dst = dst[order]
    core_bounds = np.searchsorted(dst, np.arange(0, n + 1, npc))

    percore = []
    counts_lo = np.zeros((n_cores, G), dtype=np.int64)
    counts_hi = np.zeros((n_cores, G), dtype=np.int64)
    for m in range(n_cores):
        s0, s1 = core_bounds[m], core_bounds[m + 1]
        cs = src[s0:s1]
        cd = dst[s0:s1] - m * npc
        grp = cd >> 7
        hi = (cs >= split).astype(np.int64)
        o = np.lexsort((cs, hi, grp))
        cs, cd, grp, hi = cs[o], cd[o], grp[o], hi[o]
        gb = np.searchsorted(grp * 2 + hi, np.arange(2 * G + 2))
        counts_lo[m] = gb[1::2][:G] - gb[0::2][:G]
        counts_hi[m] = gb[2::2][:G] - gb[1::2][:G]
        percore.append((cs, cd, gb))

    BGlo = np.maximum(_ceil_div(counts_lo, P).max(axis=0), 1).astype(int)
    BGhi = np.maximum(_ceil_div(counts_hi, P).max(axis=0), 1).astype(int)
    BG = (BGlo + BGhi).astype(int)
    B_total = int(BG.sum())

    # chunk schedule (uniform across cores)
    chunks = []
    t0 = 0
    ids_c = 0
    for c0 in range(0, G, CG):
        gs = list(range(c0, min(c0 + CG, G)))
        nlo = int(BGlo[gs].sum())
        nhi = int(BGhi[gs].sum())
        nblk = nlo + nhi
        lo_start, hi_start = {}, {}
        s = 0
        for g in gs:
            lo_start[g] = s
            s += int(BGlo[g])
        for g in gs:
            hi_start[g] = s
            s += int(BGhi[g])
        ch = dict(gs=gs, nlo=nlo, nhi=nhi, nblk=nblk,
                  lo_start=lo_start, hi_start=hi_start, t0=t0,
                  lo_off=ids_c, hi_off=ids_c + nlo * 8,
                  ad_off=ids_c + (nlo + nhi) * 8)
        chunks.append(ch)
        t0 += nblk
        ids_c += nblk * 8 + nblk * 8  # rows (lo+hi) + ad
    assert t0 == B_total
    ids_cols = ids_c
    CMAXB = max(ch["nblk"] for ch in chunks)

    cores = []
    for m in range(n_cores):
        cs, cd, gb = percore[m]
        dstloc = np.full((P, B_total), -1.0, dtype=np.float32)
        ids_parts = []
        for ch in chunks:
            lo_rows, hi_rows, ad_lo, ad_hi, dl_lo, dl_hi = [], [], [], [], [], []
            for g in ch["gs"]:
                for h, bgh, rows_l, ad_l, dl_l in (
                        (0, int(BGlo[g]), lo_rows, ad_lo, dl_lo),
                        (1, int(BGhi[g]), hi_rows, ad_hi, dl_hi)):
                    a, b = gb[2 * g + h], gb[2 * g + h + 1]
                    ne = b - a
                    npad = bgh * P - ne
                    assert npad >= 0
                    base = split * h
                    rows_l.append(np.concatenate(
                        [cs[a:b] - base, np.zeros(npad, np.int32)]))
                    ad_l.append(np.concatenate(
                        [cd[a:b], np.zeros(npad, np.int32)]))
                    dl_l.append(np.concatenate(
                        [(cd[a:b] - g * P).astype(np.float32),
                         np.full(npad, -1.0, np.float32)]))
            lo_flat = np.concatenate(lo_rows)
            hi_flat = np.concatenate(hi_rows)
            ad_flat = np.concatenate(ad_lo + ad_hi)
            dl_flat = np.concatenate(dl_lo + dl_hi)
            ids_parts.append(_pack_idxs(lo_flat.astype(np.int16)))
            ids_parts.append(_pack_idxs(hi_flat.astype(np.int16)))
            ids_parts.append(_pack_idxs(ad_flat.astype(np.int16)))
            nblk = ch["nblk"]
            dstloc[:, ch["t0"]:ch["t0"] + nblk] = (
                dl_flat.reshape(nblk, P).T)
        ids16 = np.tile(np.concatenate(ids_parts, axis=1), (8, 1))
        assert ids16.shape == (P, ids_cols)
        cores.append(dict(ids16=ids16, dstloc=dstloc))

    sched = dict(G=G, npc=npc, split=split, BGlo=BGlo, BGhi=BGhi, BG=BG,
                 B_total=B_total, CMAXB=CMAXB, ids_cols=ids_cols,
                 chunks=chunks)
    return sched, cores


# ---------------------------------------------------------------------------
# Weight preparation (rotation trick, head-major layout)
# ---------------------------------------------------------------------------

def _scaled_rot(a):
    """R (CxC) with row0 = a, rows 1.. orthogonal complement (f64)."""
    a = np.asarray(a, np.float64)
    nrm = np.linalg.norm(a)
    C = len(a)
    e1 = np.zeros(C); e1[0] = 1.0
    v = a / nrm - e1
    if np.linalg.norm(v) < 1e-12:
        Q = np.eye(C)
    else:
        v /= np.linalg.norm(v)
        Q = np.eye(C) - 2.0 * np.outer(v, v)
    R = Q.copy()
    R[0, :] = a
    return R


def _build_weight_ext(W1, a_src1, a_dst1, W2, a_src2, a_dst2, Wfc,
                      in_c, out_c, heads):
    H, C = heads, out_c
    HC = H * C
    f = lambda x: np.asarray(x, np.float64)
    W1, a_src1, a_dst1 = f(W1), f(a_src1), f(a_dst1)
    W2, a_src2, a_dst2 = f(W2), f(a_src2), f(a_dst2)
    Wfc = f(Wfc)

    PHM = np.zeros(HC, np.int64)   # hm col k = std col PHM[k]
    for h in range(H):
        for c in range(C):
            PHM[c * H + h] = h * C + c

    R1 = [_scaled_rot(a_src1[h]) for h in range(H)]
    R2 = [_scaled_rot(a_src2[h]) for h in range(H)]

    W1r = W1.reshape(in_c, H, C)
    W1rot = np.concatenate([W1r[:, h, :] @ R1[h].T for h in range(H)], axis=1)
    ad1cols = np.einsum("khc,hc->kh", W1r, a_dst1)
    w1ext = np.concatenate([W1rot[:, PHM], ad1cols, Wfc], axis=1)

    # B: f1rot_std -> h2 (std):  f1_true_h = inv(R1_h) applied -> fold
    B = np.zeros((HC, HC))
    for h in range(H):
        B[h * C:(h + 1) * C, :] = np.linalg.inv(R1[h]).T @ W2[h * C:(h + 1) * C, :]
    C2 = np.concatenate(
        [B[:, h * C:(h + 1) * C] @ R2[h].T for h in range(H)], axis=1)
    ad2_mat = np.zeros((HC, H))
    for h in range(H):
        ad2_mat[:, h] = B[:, h * C:(h + 1) * C] @ a_dst2[h]
    w2full = np.concatenate([C2[:, PHM], ad2_mat], axis=1)
    w2ext = w2full[PHM, :]          # rows in head-major f1 layout

    M_std = np.zeros((HC, C))
    for h in range(H):
        invR2 = np.linalg.inv(R2[h])
        M_std[h * C:(h + 1) * C, :] = invR2.T / H
    M_hm = M_std[PHM, :]

    return w1ext.astype(np.float32), dict(
        w2ext=w2ext.astype(np.float16), mmat=M_hm.astype(np.float16))


# ---------------------------------------------------------------------------
# Device program
# ---------------------------------------------------------------------------

def _build_program(sched, n, in_c, out_c, heads, add_b1, reps=1):
    G = sched["G"]
    npc = sched["npc"]
    split = sched["split"]
    BGlo, BGhi, BG = sched["BGlo"], sched["BGhi"], sched["BG"]
    B_total = sched["B_total"]
    CMAXB = sched["CMAXB"]
    ids_cols = sched["ids_cols"]
    chunks = sched["chunks"]
    H, C = heads, out_c
    HC = H * C                      # 256
    W1COLS = HC + H + out_c         # 324
    W2COLS = HC + H                 # 260

    nc = bacc.Bacc(
        "TRN2",
        target_bir_lowering=False,
        debug=False,
        enable_asserts=False,
        num_devices=N_CORES,
        num_swdge_queues=4,
    )

    xT = nc.dram_tensor("xT", [in_c, G * P], FP32, kind="ExternalInput")
    ids_d = nc.dram_tensor("ids16", [P, ids_cols], I16, kind="ExternalInput")
    dstloc_d = nc.dram_tensor("dstloc", [P, B_total], FP32,
                              kind="ExternalInput")
    w1ext_d = nc.dram_tensor("w1ext", [in_c, W1COLS], FP32,
                             kind="ExternalInput")
    w2ext_d = nc.dram_tensor("w2ext", [HC, W2COLS], FP16,
                             kind="ExternalInput")
    mmat_d = nc.dram_tensor("mmat", [HC, out_c], FP16, kind="ExternalInput")
    iota_d = nc.dram_tensor("iota", [P, P], FP16, kind="ExternalInput")
    ident_d = nc.dram_tensor("ident", [P, P], FP16, kind="ExternalInput")
    out_d = nc.dram_tensor("out", [G * P, out_c], FP32, kind="ExternalOutput")
    if DEBUG:
        t1out = nc.dram_tensor("t1out", [G * P, HC], FP16,
                               kind="ExternalOutput")
        ad1out = nc.dram_tensor("ad1out", [G * P, H], FP16,
                                kind="ExternalOutput")
        f1out = nc.dram_tensor("f1out", [G * P, HC], FP16,
                               kind="ExternalOutput")
        gtout = nc.dram_tensor("gtout", [P, 8 * HC], FP16,
                               kind="ExternalOutput")
        adgout = nc.dram_tensor("adgout", [P, 8 * H], FP16,
                                kind="ExternalOutput")
        exout = nc.dram_tensor("exout", [P, 8 * H], FP16,
                               kind="ExternalOutput")

    with tile.TileContext(nc) as tc:
        with (
            tc.tile_pool(name="const", bufs=1) as cpool,
            tc.tile_pool(name="dram", bufs=1, space="DRAM") as dpool,
        ):
            iota_t = cpool.tile([P, P], FP16)
            nc.sync.dma_start(out=iota_t[:], in_=iota_d[:])
            ident_t = cpool.tile([P, P], FP16)
            nc.sync.dma_start(out=ident_t[:], in_=ident_d[:])
            w1_t = cpool.tile([in_c, W1COLS], FP32)
            nc.sync.dma_start(out=w1_t[:], in_=w1ext_d[:])
            w2a_t = cpool.tile([P, W2COLS], FP16)
            nc.sync.dma_start(out=w2a_t[:], in_=w2ext_d[0:P, :])
            w2b_t = cpool.tile([P, W2COLS], FP16)
            nc.sync.dma_start(out=w2b_t[:], in_=w2ext_d[P:2 * P, :])
            ma_t = cpool.tile([P, out_c], FP16)
            nc.sync.dma_start(out=ma_t[:], in_=mmat_d[0:P, :])
            mb_t = cpool.tile([P, out_c], FP16)
            nc.sync.dma_start(out=mb_t[:], in_=mmat_d[P:2 * P, :])
            ids_t = cpool.tile([P, ids_cols], I16)
            nc.sync.dma_start(out=ids_t[:], in_=ids_d[:])
            dstloc_t = cpool.tile([P, B_total], FP32)
            nc.sync.dma_start(out=dstloc_t[:], in_=dstloc_d[:])

            f1_sb = cpool.tile([P, G * HC], FP16)
            xch_sb = cpool.tile([P, G * out_c], FP32)

            for _rep in range(reps):
              table1_own = dpool.tile([npc, HC], FP16, tag=f"t1o{_rep}",
                                      name=f"table1_own{_rep}")
              table1 = dpool.tile([n, HC], FP16, addr_space="Shared",
                                  tag=f"t1{_rep}", name=f"table1{_rep}")
              table2_own = dpool.tile([npc, HC], FP16, tag=f"t2o{_rep}",
                                      name=f"table2_own{_rep}")
              table2 = dpool.tile([n, HC], FP16, addr_space="Shared",
                                  tag=f"t2{_rep}", name=f"table2{_rep}")
              ad1_dram = dpool.tile([npc, P], FP16, tag=f"a1{_rep}",
                                    name=f"ad1_dram{_rep}")
              ad2_dram = dpool.tile([npc, P], FP16, tag=f"a2{_rep}",
                                    name=f"ad2_dram{_rep}")

              # ---------------- Phase A: layer-1 node transform --------------
              with (
                  tc.tile_pool(name=f"pa{_rep}", bufs=3) as pa,
                  tc.tile_pool(name=f"pa_ps{_rep}", bufs=2, space="PSUM") as pa_ps,
              ):
                  for g in range(G):
                      xt_t = pa.tile([in_c, P], FP32, tag="xt")
                      nc.sync.dma_start(out=xt_t[:], in_=xT[:, g * P:(g + 1) * P])
                      ph = pa_ps.tile([P, W1COLS], FP32, tag="ph")
                      nc.tensor.matmul(ph[:], lhsT=xt_t[:], rhs=w1_t[:],
                                       start=True, stop=True)
                      rows = min(P, npc - g * P)
                      tx = pa.tile([P, HC], FP16, tag="tx")
                      nc.scalar.copy(tx[:], ph[:, 0:HC])
                      adx = pa.tile([P, H], FP16, tag="adx")
                      nc.vector.tensor_copy(adx[:], ph[:, HC:HC + H])
                      nc.vector.tensor_copy(
                          xch_sb[:, g * out_c:(g + 1) * out_c],
                          ph[:, HC + H:W1COLS])
                      nc.sync.dma_start(
                          out=table1_own[g * P:g * P + rows, :],
                          in_=tx[:rows, :])
                      nc.sync.dma_start(
                          out=ad1_dram[g * P:g * P + rows, 0:H],
                          in_=adx[:rows, :])
                      if DEBUG and _rep == 0:
                          nc.sync.dma_start(
                              out=t1out[g * P:(g + 1) * P, :], in_=tx[:])
                          nc.sync.dma_start(
                              out=ad1out[g * P:(g + 1) * P, :], in_=adx[:])

              if not SKIP_AG:
                  nc.gpsimd.collective_compute(
                      "AllGather",
                      mybir.AluOpType.bypass,
                      replica_groups=[list(range(N_CORES))],
                      ins=[table1_own[:].opt()],
                      outs=[table1[:].opt()],
                  )

              # ---------------- Edge phase ----------------
              def edge_phase(table, ad_dram, layer):
                  first = str(layer).endswith("_1")
                  with (
                      tc.tile_pool(name=f"gt{layer}", bufs=2) as gpool,
                      tc.tile_pool(name=f"ad{layer}", bufs=2) as adpool,
                      tc.tile_pool(name=f"ez{layer}", bufs=2) as ezpool,
                      tc.tile_pool(name=f"sS{layer}", bufs=8) as spool,
                      tc.tile_pool(name=f"ev{layer}", bufs=3) as evpool,
                      tc.tile_pool(name=f"pso{layer}", bufs=2, space="PSUM") as pso,
                      tc.tile_pool(name=f"pst{layer}", bufs=2, space="PSUM") as pst,
                      tc.tile_pool(name=f"psf{layer}", bufs=2, space="PSUM") as psf,
                  ):
                      qn = 0
                      for ch in chunks:
                          nblkc = ch["nblk"]
                          gt = gpool.tile([P, CMAXB * HC], FP16, tag="g")
                          gt3 = gt[:].rearrange("p (c e) -> p c e", e=HC)
                          if ch["nlo"]:
                              nc.gpsimd.dma_gather(
                                  gt3[:, 0:ch["nlo"], :], table[0:split, :],
                                  ids_t[:, ch["lo_off"]:ch["lo_off"] + ch["nlo"] * 8],
                                  ch["nlo"] * P, ch["nlo"] * P, HC,
                                  single_packet=False, queue_num=qn % 4)
                              qn += 1
                          if ch["nhi"]:
                              nc.gpsimd.dma_gather(
                                  gt3[:, ch["nlo"]:nblkc, :], table[split:n, :],
                                  ids_t[:, ch["hi_off"]:ch["hi_off"] + ch["nhi"] * 8],
                                  ch["nhi"] * P, ch["nhi"] * P, HC,
                                  single_packet=False, queue_num=qn % 4)
                              qn += 1
                          adg = adpool.tile([P, CMAXB * P], FP16, tag="a")
                          adg3 = adg[:].rearrange("p (c e) -> p c e", e=P)
                          nc.gpsimd.dma_gather(
                              adg3[:, 0:nblkc, :], ad_dram[:],
                              ids_t[:, ch["ad_off"]:ch["ad_off"] + nblkc * 8],
                              nblkc * P, nblkc * P, P,
                              single_packet=False, queue_num=qn % 4)
                          qn += 1

                          if GATHER_ONLY:
                              continue

                          # z = as + ad ; ex = exp(lrelu(z))   (chunk-batched)
                          z4 = ezpool.tile([P, CMAXB * H], FP16, tag="z")
                          z43 = z4[:].rearrange("p (c e) -> p c e", e=H)
                          nc.vector.tensor_tensor(
                              out=z43[:, 0:nblkc, :],
                              in0=gt3[:, 0:nblkc, 0:H],
                              in1=adg3[:, 0:nblkc, 0:H],
                              op=mybir.AluOpType.add)
                          lr4 = ezpool.tile([P, CMAXB * H], FP16, tag="l")
                          nc.vector.tensor_scalar(
                              lr4[:, 0:nblkc * H], z4[:, 0:nblkc * H],
                              NEG_SLOPE, None, mybir.AluOpType.mult)
                          nc.vector.tensor_tensor(
                              out=z4[:, 0:nblkc * H], in0=z4[:, 0:nblkc * H],
                              in1=lr4[:, 0:nblkc * H], op=mybir.AluOpType.max)
                          ex4 = ezpool.tile([P, CMAXB * H], FP16, tag="e")
                          nc.scalar.activation(
                              ex4[:, 0:nblkc * H], z4[:, 0:nblkc * H],
                              mybir.ActivationFunctionType.Exp)
                          # messages *= ex (broadcast over the 64 c-positions)
                          gt4 = gt[:].rearrange("p (c j h) -> p c j h",
                                                j=C, h=H)
                          ex4v = ex4[:].rearrange("p (c e) -> p c e", e=H)
                          ex4b = bass.AP(
                              ex4v.tensor, ex4v.offset,
                              [ex4v.ap[0], [H, nblkc], [0, C], [1, H]])
                          if DEBUG and first and ch["t0"] == 0:
                              nc.sync.dma_start(out=gtout[:],
                                                in_=gt[:, 0:8 * HC])
                              nc.sync.dma_start(
                                  out=adgout[:],
                                  in_=bass.AP(adg[:].tensor, adg[:].offset,
                                              [adg[:].ap[0], [P, 8], [1, H]]))
                              nc.sync.dma_start(out=exout[:],
                                                in_=ex4[:, 0:8 * H])
                          nc.vector.tensor_tensor(
                              out=gt4[:, 0:nblkc, :, :],
                              in0=gt4[:, 0:nblkc, :, :],
                              in1=ex4b, op=mybir.AluOpType.mult)

                          for g in ch["gs"]:
                              blocks = (
                                  list(range(ch["lo_start"][g],
                                             ch["lo_start"][g] + int(BGlo[g])))
                                  + list(range(ch["hi_start"][g],
                                               ch["hi_start"][g] + int(BGhi[g]))))
                              ps = pso.tile([P, HC], FP32, tag="po")
                              pden = pso.tile([P, H], FP32, tag="pd")
                              nb = len(blocks)
                              for j, slot in enumerate(blocks):
                                  S = spool.tile([P, P], FP16, tag="S")
                                  nc.vector.tensor_scalar(
                                      S[:], iota_t[:],
                                      dstloc_t[:, ch["t0"] + slot:
                                               ch["t0"] + slot + 1],
                                      None, mybir.AluOpType.is_equal)
                                  nc.tensor.matmul(
                                      ps[:], lhsT=S[:],
                                      rhs=gt3[:, slot, :],
                                      start=(j == 0), stop=(j == nb - 1))
                                  nc.tensor.matmul(
                                      pden[:], lhsT=S[:],
                                      rhs=ex4v[:, slot, :],
                                      start=(j == 0), stop=(j == nb - 1))

                              # ---- evacuate group ----
                              rec = evpool.tile([P, H], FP32, tag="rec")
                              nc.vector.tensor_scalar(
                                  rec[:], pden[:], 1e-16, None,
                                  mybir.AluOpType.add)
                              nc.vector.reciprocal(rec[:], rec[:])
                              recb = bass.AP(
                                  rec[:].tensor, rec[:].offset,
                                  [rec[:].ap[0], [0, C], [1, H]])
                              if first:
                                  nc.vector.tensor_tensor(
                                      out=f1_sb[:, g * HC:(g + 1) * HC],
                                      in0=ps[:], in1=recb,
                                      op=mybir.AluOpType.mult)
                                  if DEBUG and str(layer) == "0_1":
                                      nc.sync.dma_start(
                                          out=f1out[g * P:(g + 1) * P, :],
                                          in_=f1_sb[:, g * HC:(g + 1) * HC])
                              else:
                                  tmp = evpool.tile([P, HC], FP16, tag="tmp")
                                  nc.vector.tensor_tensor(
                                      out=tmp[:], in0=ps[:], in1=recb,
                                      op=mybir.AluOpType.mult)
                                  pf = psf.tile([P, out_c], FP32, tag="pf")
                                  for k in range(2):
                                      pt = pst.tile([P, P], FP16, tag="pt")
                                      nc.tensor.transpose(
                                          pt[:], tmp[:, k * P:(k + 1) * P],
                                          ident_t[:])
                                      fT = evpool.tile([P, P], FP16, tag="fT")
                                      nc.scalar.copy(fT[:], pt[:])
                                      nc.tensor.matmul(
                                          pf[:], lhsT=fT[:],
                                          rhs=(ma_t if k == 0 else mb_t)[:],
                                          start=(k == 0), stop=(k == 1))
                                  ob = evpool.tile([P, out_c], FP32, tag="ob")
                                  nc.vector.tensor_tensor(
                                      out=ob[:], in0=pf[:],
                                      in1=xch_sb[:, g * out_c:(g + 1) * out_c],
                                      op=mybir.AluOpType.add)
                                  nc.sync.dma_start(
                                      out=out_d[g * P:(g + 1) * P, :],
                                      in_=ob[:])

              if SKIP_EDGE or GATHER_ONLY:
                  nc.vector.memset(f1_sb[:], 0.0)
              if not (SKIP_EDGE or GATHER_ONLY):
                  edge_phase(table1, ad1_dram, layer=f"{_rep}_1")
              elif GATHER_ONLY:
                  edge_phase(table1, ad1_dram, layer=f"{_rep}_1")

              # ---------------- Phase D: layer-2 node transform --------------
              with (
                  tc.tile_pool(name=f"pd{_rep}", bufs=3) as pd,
                  tc.tile_pool(name=f"pd_ps{_rep}", bufs=2, space="PSUM") as pd_ps,
                  tc.tile_pool(name=f"pd_pt{_rep}", bufs=2, space="PSUM") as pd_pt,
              ):
                  for g in range(G):
                      ph = pd_ps.tile([P, W2COLS], FP32, tag="ph2")
                      for k in range(2):
                          pft = pd_pt.tile([P, P], FP16, tag="pft")
                          nc.tensor.transpose(
                              pft[:],
                              f1_sb[:, g * HC + k * P:g * HC + (k + 1) * P],
                              ident_t[:])
                          fT = pd.tile([P, P], FP16, tag="fT")
                          nc.scalar.copy(fT[:], pft[:])
                          nc.tensor.matmul(
                              ph[:], lhsT=fT[:],
                              rhs=(w2a_t if k == 0 else w2b_t)[:],
                              start=(k == 0), stop=(k == 1))
                      rows = min(P, npc - g * P)
                      tx = pd.tile([P, HC], FP16, tag="tx2")
                      nc.scalar.copy(tx[:], ph[:, 0:HC])
                      adx = pd.tile([P, H], FP16, tag="adx2")
                      nc.vector.tensor_copy(adx[:], ph[:, HC:W2COLS])
                      nc.sync.dma_start(
                          out=table2_own[g * P:g * P + rows, :],
                          in_=tx[:rows, :])
                      nc.sync.dma_start(
                          out=ad2_dram[g * P:g * P + rows, 0:H],
                          in_=adx[:rows, :])

              if not SKIP_AG:
                  nc.gpsimd.collective_compute(
                      "AllGather",
                      mybir.AluOpType.bypass,
                      replica_groups=[list(range(N_CORES))],
                      ins=[table2_own[:].opt()],
                      outs=[table2[:].opt()],
                  )

              if not (SKIP_EDGE or GATHER_ONLY):
                  edge_phase(table2, ad2_dram, layer=f"{_rep}_2")
              else:
                  if GATHER_ONLY:
                      edge_phase(table2, ad2_dram, layer=f"{_rep}_2")
                  ob0 = cpool.tile([P, out_c], FP32)
                  nc.vector.memset(ob0[:], 0.0)
                  for g in range(G):
                      nc.sync.dma_start(out=out_d[g * P:(g + 1) * P, :],
                                        in_=ob0[:])

    nc.compile()
    return nc


# ---------------------------------------------------------------------------
# Entry point
# ---------------------------------------------------------------------------

def _build_in_maps(x, sched, cores, w1ext, w2d, b1, add_b1):
    G, npc = sched["G"], sched["npc"]
    n, in_c = x.shape
    iota = np.broadcast_to(np.arange(P, dtype=np.float16), (P, P)).copy()
    ident = np.eye(P, dtype=np.float16)
    in_maps = []
    for m in range(N_CORES):
        xpad = np.zeros((G * P, in_c), dtype=np.float32)
        xpad[:npc] = x[m * npc:(m + 1) * npc]
        im = dict(
            xT=np.ascontiguousarray(xpad.T),
            ids16=cores[m]["ids16"],
            dstloc=cores[m]["dstloc"],
            w1ext=w1ext,
            w2ext=w2d["w2ext"],
            mmat=w2d["mmat"],
            iota=iota,
            ident=ident,
        )
        in_maps.append(im)
    return in_maps


def kernel(x, edge_index, W1, a_src1, a_dst1, b1, W2, a_src2, a_dst2, b2,
           Wfc, bfc):
    x = np.asarray(x, dtype=np.float32)
    n, in_c = x.shape
    heads, out_c = np.asarray(a_src1).shape

    sched, cores = _preprocess(edge_index, n, N_CORES)
    npc = sched["npc"]

    w1ext, w2d = _build_weight_ext(W1, a_src1, a_dst1, W2, a_src2, a_dst2,
                                   Wfc, in_c, out_c, heads)
    b1 = np.asarray(b1, np.float64)
    if np.any(b1 != 0):
        # fold b1 through layer 2 on host is possible but unused for this
        # problem (b1 == 0); fall back to adding its linearized effect.
        raise NotImplementedError("b1 != 0 not supported")
    nc = _build_program(sched, n, in_c, out_c, heads, False)
    in_maps = _build_in_maps(x, sched, cores, w1ext, w2d, b1, False)

    res = run_bass_kernel_spmd(nc, in_maps, list(range(N_CORES)))
    global LAST_RESULTS
    LAST_RESULTS = res
    outs = [res.results[m]["out"][:npc] for m in range(N_CORES)]
    out = np.concatenate(outs, axis=0)
    out = out + (np.asarray(b2) + np.asarray(bfc))[None, :]
    return out.astype(np.float32)


# ---------------------------------------------------------------------------
# Numpy emulation of the device math (for host-side validation)
# ---------------------------------------------------------------------------

def _emulate(x, edge_index, w1ext, w2d, fp16=True):
    cast = (lambda a: a.astype(np.float16).astype(np.float32)) if fp16 else \
           (lambda a: a)
    x = np.asarray(x, np.float32)
    n = x.shape[0]
    H, C = HEADS, OUT_C
    loops = np.arange(n, dtype=np.int64)
    src = np.concatenate([np.asarray(edge_index[0]), loops])
    dst = np.concatenate([np.asarray(edge_index[1]), loops])

    def lay(table_hm, adn, w_ext=None):
        as_e = table_hm[src][:, :H]
        z = as_e + adn[dst]
        e = np.where(z >= 0, z, NEG_SLOPE * z)
        ex = cast(np.exp(e))
        s = np.zeros((n, H)); np.add.at(s, dst, ex)
        alpha = ex / (s[dst] + 1e-16)
        msg = cast(table_hm[src].reshape(-1, C, H) * alpha[:, None, :])
        f = np.zeros((n, C, H)); np.add.at(f, dst, msg)
        return f.reshape(n, H * C)

    t1 = cast(x @ w1ext[:, :256])
    ad1 = x @ w1ext[:, 256:260]
    xch = x @ w1ext[:, 260:324]
    f1 = cast(lay(t1, ad1))
    w2 = w2d["w2ext"].astype(np.float32)
    t2 = cast(f1 @ w2[:, :256])
    ad2 = f1 @ w2[:, 256:260]
    f2 = lay(t2, ad2)
    return f2 @ w2d["mmat"].astype(np.float32) + xch


# revision 17
# speedup vs baseline: 1.5613x; 1.5613x over previous
"""Trainium2 Bass kernel for a 2-layer GAT block (gnn_message_passing).

v3: dense dst-major edge blocks.
  - fp16 node tables, rows exactly 512B (256 halves, head-major channel
    layout c*H+h).  Householder rotation trick: alpha_src per edge is
    channels 0..3 of the gathered row; inverse rotations folded into W2 /
    the final mean-over-heads matmul on the host.
  - Edges sorted by dst, sharded by dst range, grouped into 128-dst groups.
    Within each (group, src-half): DENSE dst-major packing -- slot
    (partition p, block j) holds the j-th edge of local dst p (idx-0 pad +
    fp16 mask).  Segment-sum = identity-matmul PSUM accumulation (NO one-hot
    S build), a_dst add is a per-partition SBUF broadcast (NO per-edge ad
    gather), softmax denominators via batched tensor_reduce.  Edges beyond
    the per-(group,half) cap K go to small dst-sorted TAIL blocks using the
    one-hot-S route with a 256B/edge ad gather.
  - Groups processed in chunks of CG; one dma_gather call per (chunk, half)
    plus tiny tail-ad calls.
"""

import numpy as np

import concourse.bass as bass
import concourse.bacc as bacc
import concourse.mybir as mybir
import concourse.tile as tile
from concourse.bass_utils import run_bass_kernel_spmd

N = 50000
E = 800000
IN_C = 128
OUT_C = 64
HEADS = 4
NEG_SLOPE = 0.2
N_CORES = 8

P = 128
CG = 2              # groups per gather chunk

FP32 = mybir.dt.float32
FP16 = mybir.dt.float16
I16 = mybir.dt.int16

# timing-triage flags
SKIP_EDGE = False
SKIP_AG = False
GATHER_ONLY = False
DEBUG_F1 = False


def _ceil_div(a, b):
    return (a + b - 1) // b


def _pack_idxs(flat):
    m = len(flat)
    assert m % 16 == 0
    arr = np.zeros((16, m // 16), np.int16)
    arr[np.arange(m) % 16, np.arange(m) // 16] = flat
    return arr


# ---------------------------------------------------------------------------
# Host-side preprocessing
# ---------------------------------------------------------------------------

def _preprocess(edge_index, n, n_cores):
    npc = n // n_cores
    G = _ceil_div(npc, P)
    split = (n + 1) // 2

    src = np.asarray(edge_index[0], dtype=np.int64)
    dst = np.asarray(edge_index[1], dtype=np.int64)
    loops = np.arange(n, dtype=np.int64)
    src = np.concatenate([src, loops]).astype(np.int32)
    dst = np.concatenate([dst, loops]).astype(np.int32)

    order = np.argsort(dst, kind="stable")
    src = src[order]
    dst = dst[order]
    core_bounds = np.searchsorted(dst, np.arange(0, n + 1, npc))

    percore = []
    deg = np.zeros((n_cores, G, 2, P), np.int64)
    for m in range(n_cores):
        s0, s1 = core_bounds[m], core_bounds[m + 1]
        cs = src[s0:s1]
        cd = dst[s0:s1] - m * npc
        grp = cd >> 7
        hi = (cs >= split).astype(np.int64)
        o = np.lexsort((cs, cd, hi, grp))
        cs, cd, grp, hi = cs[o], cd[o], grp[o], hi[o]
        np.add.at(deg, (m, grp, hi, cd & 127), 1)
        gb = np.searchsorted(grp * 2 + hi, np.arange(2 * G + 2))
        percore.append((cs, cd, gb))

    # per-(group,half) dense cap K and uniform tail block count
    K = np.zeros((G, 2), np.int64)
    TB = np.zeros((G, 2), np.int64)
    for g in range(G):
        for h in range(2):
            d = deg[:, g, h, :]
            dmax = int(d.max())
            best = None
            for k in range(0, dmax + 1):
                tail = int(np.maximum(d - k, 0).sum(axis=1).max())
                tb = _ceil_div(tail, P)
                cost = P * k + 2 * P * tb + 64 * (k + tb)
                if best is None or cost < best[0]:
                    best = (cost, k, tb)
            K[g, h] = best[1]
            TB[g, h] = best[2]

    chunks = []
    t0 = 0
    tt0 = 0
    ids_c = 0
    for c0 in range(0, G, CG):
        gs = list(range(c0, min(c0 + CG, G)))
        sec = {}
        s = 0
        for g in gs:
            sec[("dl", g)] = (s, int(K[g, 0])); s += int(K[g, 0])
        tlo_start = s
        for g in gs:
            sec[("tl", g)] = (s, int(TB[g, 0])); s += int(TB[g, 0])
        nlo = s
        for g in gs:
            sec[("dh", g)] = (s, int(K[g, 1])); s += int(K[g, 1])
        thi_start = s
        for g in gs:
            sec[("th", g)] = (s, int(TB[g, 1])); s += int(TB[g, 1])
        nblk = s
        nhi = nblk - nlo
        n_tlo = sum(int(TB[g, 0]) for g in gs)
        n_thi = sum(int(TB[g, 1]) for g in gs)
        ch = dict(gs=gs, sec=sec, nlo=nlo, nhi=nhi, nblk=nblk, t0=t0,
                  tlo_start=tlo_start, thi_start=thi_start,
                  n_tlo=n_tlo, n_thi=n_thi, tt0=tt0,
                  lo_off=ids_c, hi_off=ids_c + nlo * 8,
                  adlo_off=ids_c + nblk * 8,
                  adhi_off=ids_c + (nblk + n_tlo) * 8)
        chunks.append(ch)
        t0 += nblk
        tt0 += n_tlo + n_thi
        ids_c += (nblk + n_tlo + n_thi) * 8
    B_total = t0
    TB_total = tt0
    ids_cols = ids_c
    CMAXB = max(ch["nblk"] for ch in chunks)
    TMAXB = max((ch["n_tlo"] + ch["n_thi"]) for ch in chunks)

    cores = []
    for m in range(n_cores):
        cs, cd, gb = percore[m]
        mask = np.zeros((P, B_total), np.float16)
        dstloc = np.full((P, max(TB_total, 1)), -1.0, dtype=np.float32)
        ids_parts = []
        tcol = 0
        for ch in chunks:
            idx_slot = np.zeros((P, ch["nblk"]), np.int32)
            ad_lo_l, ad_hi_l = [], []
            for half_pass in (0, 1):
                for g in ch["gs"]:
                    h = half_pass
                    dkey = "dl" if h == 0 else "dh"
                    tkey = "tl" if h == 0 else "th"
                    ad_l = ad_lo_l if h == 0 else ad_hi_l
                    a, b = gb[2 * g + h], gb[2 * g + h + 1]
                    ecs = cs[a:b] - split * h
                    dloc = cd[a:b] & 127
                    k = int(K[g, h])
                    ds, dn = ch["sec"][(dkey, g)]
                    runpos = (np.arange(len(dloc))
                              - np.searchsorted(dloc, dloc, side="left"))
                    dense_sel = runpos < k
                    if k:
                        idx_slot[dloc[dense_sel],
                                 ds + runpos[dense_sel]] = ecs[dense_sel]
                        mk = np.zeros((P, k), np.float16)
                        mk[dloc[dense_sel], runpos[dense_sel]] = 1.0
                        mask[:, ch["t0"] + ds:ch["t0"] + ds + k] = mk
                    ts_, tn = ch["sec"][(tkey, g)]
                    if tn:
                        tsel = ~dense_sel
                        tcs, tcd = ecs[tsel], dloc[tsel]
                        npad = tn * P - len(tcs)
                        assert npad >= 0, (g, h, tn, len(tcs))
                        tflat = np.concatenate(
                            [tcs, np.zeros(npad, np.int32)])
                        tdl = np.concatenate(
                            [tcd.astype(np.float32),
                             np.full(npad, -1.0, np.float32)])
                        tad = np.concatenate(
                            [(tcd + g * P).astype(np.int32),
                             np.zeros(npad, np.int32)])
                        idx_slot[:, ts_:ts_ + tn] = tflat.reshape(tn, P).T
                        mask[:, ch["t0"] + ts_:ch["t0"] + ts_ + tn] = 1.0
                        dstloc[:, tcol:tcol + tn] = tdl.reshape(tn, P).T
                        tcol += tn
                        ad_l.append(tad)
                    else:
                        assert (~dense_sel).sum() == 0
            # phantom dsts (zero edges in this group, e.g. node ids
            # beyond npc in the last group): force one masked-in pad slot
            # so their softmax denominator is nonzero (avoids 0*inf NaNs
            # that the identity-matmul would spread across partitions).
            for g in ch["gs"]:
                dtot = deg[m, g].sum(axis=0)          # [P]
                ph_d = np.where(dtot == 0)[0]
                if len(ph_d):
                    for dkey in ("dl", "dh"):
                        ds, dn = ch["sec"][(dkey, g)]
                        if dn:
                            mask[ph_d, ch["t0"] + ds] = 1.0
                            break
                    else:
                        raise AssertionError("group with no dense slots")
            lo_flat = idx_slot[:, 0:ch["nlo"]].T.ravel()
            hi_flat = idx_slot[:, ch["nlo"]:ch["nblk"]].T.ravel()
            ids_parts.append(_pack_idxs(lo_flat.astype(np.int16)))
            ids_parts.append(_pack_idxs(hi_flat.astype(np.int16)))
            ad_flat = (np.concatenate(ad_lo_l + ad_hi_l)
                       if (ad_lo_l or ad_hi_l) else np.zeros(0, np.int32))
            ids_parts.append(_pack_idxs(ad_flat.astype(np.int16)))
        assert tcol == TB_total
        ids16 = np.tile(np.concatenate(ids_parts, axis=1), (8, 1))
        assert ids16.shape == (P, ids_cols), (ids16.shape, ids_cols)
        cores.append(dict(ids16=ids16, dstloc=dstloc, mask=mask))

    sched = dict(G=G, npc=npc, split=split, K=K, TB=TB,
                 B_total=B_total, TB_total=TB_total, CMAXB=CMAXB,
                 TMAXB=TMAXB, ids_cols=ids_cols, chunks=chunks)
    return sched, cores


# ---------------------------------------------------------------------------
# Weight preparation (rotation trick, head-major layout)
# ---------------------------------------------------------------------------

def _scaled_rot(a):
    a = np.asarray(a, np.float64)
    nrm = np.linalg.norm(a)
    C = len(a)
    e1 = np.zeros(C); e1[0] = 1.0
    v = a / nrm - e1
    if np.linalg.norm(v) < 1e-12:
        Q = np.eye(C)
    else:
        v /= np.linalg.norm(v)
        Q = np.eye(C) - 2.0 * np.outer(v, v)
    R = Q.copy()
    R[0, :] = a
    return R


def _build_weight_ext(W1, a_src1, a_dst1, W2, a_src2, a_dst2, Wfc,
                      in_c, out_c, heads):
    H, C = heads, out_c
    HC = H * C
    f = lambda x: np.asarray(x, np.float64)
    W1, a_src1, a_dst1 = f(W1), f(a_src1), f(a_dst1)
    W2, a_src2, a_dst2 = f(W2), f(a_src2), f(a_dst2)
    Wfc = f(Wfc)

    PHM = np.zeros(HC, np.int64)
    for h in range(H):
        for c in range(C):
            PHM[c * H + h] = h * C + c

    R1 = [_scaled_rot(a_src1[h]) for h in range(H)]
    R2 = [_scaled_rot(a_src2[h]) for h in range(H)]

    W1r = W1.reshape(in_c, H, C)
    W1rot = np.concatenate([W1r[:, h, :] @ R1[h].T for h in range(H)], axis=1)
    ad1cols = np.einsum("khc,hc->kh", W1r, a_dst1)
    w1ext = np.concatenate([W1rot[:, PHM], ad1cols, Wfc], axis=1)

    B = np.zeros((HC, HC))
    for h in range(H):
        B[h * C:(h + 1) * C, :] = \
            np.linalg.inv(R1[h]).T @ W2[h * C:(h + 1) * C, :]
    C2 = np.concatenate(
        [B[:, h * C:(h + 1) * C] @ R2[h].T for h in range(H)], axis=1)
    ad2_mat = np.zeros((HC, H))
    for h in range(H):
        ad2_mat[:, h] = B[:, h * C:(h + 1) * C] @ a_dst2[h]
    w2full = np.concatenate([C2[:, PHM], ad2_mat], axis=1)
    w2ext = w2full[PHM, :]

    M_std = np.zeros((HC, C))
    for h in range(H):
        M_std[h * C:(h + 1) * C, :] = np.linalg.inv(R2[h]).T / H
    M_hm = M_std[PHM, :]

    return w1ext.astype(np.float32), dict(
        w2ext=w2ext.astype(np.float16), mmat=M_hm.astype(np.float16))


# ---------------------------------------------------------------------------
# Device program
# ---------------------------------------------------------------------------

def _build_program(sched, n, in_c, out_c, heads, add_b1, reps=1):
    G = sched["G"]
    npc = sched["npc"]
    split = sched["split"]
    K, TB = sched["K"], sched["TB"]
    B_total = sched["B_total"]
    TB_total = sched["TB_total"]
    CMAXB = sched["CMAXB"]
    TMAXB = sched["TMAXB"]
    ids_cols = sched["ids_cols"]
    chunks = sched["chunks"]
    H, C = heads, out_c
    HC = H * C
    W1COLS = HC + H + out_c
    W2COLS = HC + H

    nc = bacc.Bacc(
        "TRN2",
        target_bir_lowering=False,
        debug=False,
        enable_asserts=False,
        num_devices=N_CORES,
        num_swdge_queues=4,
    )

    xT = nc.dram_tensor("xT", [in_c, G * P], FP32, kind="ExternalInput")
    ids_d = nc.dram_tensor("ids16", [P, ids_cols], I16, kind="ExternalInput")
    dstloc_d = nc.dram_tensor("dstloc", [P, max(TB_total, 1)], FP32,
                              kind="ExternalInput")
    mask_d = nc.dram_tensor("mask", [P, B_total], FP16, kind="ExternalInput")
    w1ext_d = nc.dram_tensor("w1ext", [in_c, W1COLS], FP32,
                             kind="ExternalInput")
    w2ext_d = nc.dram_tensor("w2ext", [HC, W2COLS], FP16,
                             kind="ExternalInput")
    mmat_d = nc.dram_tensor("mmat", [HC, out_c], FP16, kind="ExternalInput")
    iota_d = nc.dram_tensor("iota", [P, P], FP16, kind="ExternalInput")
    ident_d = nc.dram_tensor("ident", [P, P], FP16, kind="ExternalInput")
    out_d = nc.dram_tensor("out", [G * P, out_c], FP32, kind="ExternalOutput")
    if DEBUG_F1:
        f1out = nc.dram_tensor("f1out", [G * P, HC], FP16,
                               kind="ExternalOutput")
        t1out = nc.dram_tensor("t1out", [G * P, HC], FP16,
                               kind="ExternalOutput")
        ad1out = nc.dram_tensor("ad1out", [G * P, H], FP16,
                                kind="ExternalOutput")
        t2out = nc.dram_tensor("t2out", [G * P, HC], FP16,
                               kind="ExternalOutput")
        ad2out = nc.dram_tensor("ad2out", [G * P, H], FP16,
                                kind="ExternalOutput")

    with tile.TileContext(nc) as tc:
        with (
            tc.tile_pool(name="const", bufs=1) as cpool,
            tc.tile_pool(name="dram", bufs=1, space="DRAM") as dpool,
        ):
            iota_t = cpool.tile([P, P], FP16)
            nc.sync.dma_start(out=iota_t[:], in_=iota_d[:])
            ident_t = cpool.tile([P, P], FP16)
            nc.sync.dma_start(out=ident_t[:], in_=ident_d[:])
            w1_t = cpool.tile([in_c, W1COLS], FP32)
            nc.sync.dma_start(out=w1_t[:], in_=w1ext_d[:])
            w2a_t = cpool.tile([P, W2COLS], FP16)
            nc.sync.dma_start(out=w2a_t[:], in_=w2ext_d[0:P, :])
            w2b_t = cpool.tile([P, W2COLS], FP16)
            nc.sync.dma_start(out=w2b_t[:], in_=w2ext_d[P:2 * P, :])
            ma_t = cpool.tile([P, out_c], FP16)
            nc.sync.dma_start(out=ma_t[:], in_=mmat_d[0:P, :])
            mb_t = cpool.tile([P, out_c], FP16)
            nc.sync.dma_start(out=mb_t[:], in_=mmat_d[P:2 * P, :])
            ids_t = cpool.tile([P, ids_cols], I16)
            nc.sync.dma_start(out=ids_t[:], in_=ids_d[:])
            dstloc_t = cpool.tile([P, max(TB_total, 1)], FP32)
            nc.sync.dma_start(out=dstloc_t[:], in_=dstloc_d[:])
            mask_t = cpool.tile([P, B_total], FP16)
            nc.sync.dma_start(out=mask_t[:], in_=mask_d[:])

            f1_sb = cpool.tile([P, G * HC], FP16)
            xch_sb = cpool.tile([P, G * out_c], FP32)
            ad_sb1 = cpool.tile([P, G * H], FP16)
            ad_sb2 = cpool.tile([P, G * H], FP16)

            for _rep in range(reps):
              table1_own = dpool.tile([npc, HC], FP16, tag=f"t1o{_rep}",
                                      name=f"table1_own{_rep}")
              table1 = dpool.tile([n, HC], FP16, addr_space="Shared",
                                  tag=f"t1{_rep}", name=f"table1{_rep}")
              table2_own = dpool.tile([npc, HC], FP16, tag=f"t2o{_rep}",
                                      name=f"table2_own{_rep}")
              table2 = dpool.tile([n, HC], FP16, addr_space="Shared",
                                  tag=f"t2{_rep}", name=f"table2{_rep}")
              ad1_dram = dpool.tile([npc, P], FP16, tag=f"a1{_rep}",
                                    name=f"ad1_dram{_rep}")
              ad2_dram = dpool.tile([npc, P], FP16, tag=f"a2{_rep}",
                                    name=f"ad2_dram{_rep}")

              # ---------------- Phase A ----------------
              with (
                  tc.tile_pool(name=f"pa{_rep}", bufs=3) as pa,
                  tc.tile_pool(name=f"pa_ps{_rep}", bufs=2, space="PSUM") as pa_ps,
              ):
                  for g in range(G):
                      xt_t = pa.tile([in_c, P], FP32, tag="xt")
                      nc.sync.dma_start(out=xt_t[:],
                                        in_=xT[:, g * P:(g + 1) * P])
                      ph = pa_ps.tile([P, W1COLS], FP32, tag="ph")
                      nc.tensor.matmul(ph[:], lhsT=xt_t[:], rhs=w1_t[:],
                                       start=True, stop=True)
                      rows = min(P, npc - g * P)
                      tx = pa.tile([P, HC], FP16, tag="tx")
                      nc.scalar.copy(tx[:], ph[:, 0:HC])
                      nc.vector.tensor_copy(ad_sb1[:, g * H:(g + 1) * H],
                                            ph[:, HC:HC + H])
                      nc.vector.tensor_copy(
                          xch_sb[:, g * out_c:(g + 1) * out_c],
                          ph[:, HC + H:W1COLS])
                      nc.sync.dma_start(
                          out=table1_own[g * P:g * P + rows, :],
                          in_=tx[:rows, :])
                      if TB.sum():
                          nc.sync.dma_start(
                              out=ad1_dram[g * P:g * P + rows, 0:H],
                              in_=ad_sb1[:rows, g * H:(g + 1) * H])
                      if DEBUG_F1 and _rep == 0:
                          nc.sync.dma_start(
                              out=t1out[g * P:(g + 1) * P, :], in_=tx[:])
                          nc.sync.dma_start(
                              out=ad1out[g * P:(g + 1) * P, :],
                              in_=ad_sb1[:, g * H:(g + 1) * H])

              if not SKIP_AG:
                  nc.gpsimd.collective_compute(
                      "AllGather",
                      mybir.AluOpType.bypass,
                      replica_groups=[list(range(N_CORES))],
                      ins=[table1_own[:].opt()],
                      outs=[table1[:].opt()],
                  )

              # ---------------- Edge phase ----------------
              def edge_phase(table, ad_dram, ad_sb, layer):
                  first = str(layer).endswith("_1")
                  with (
                      tc.tile_pool(name=f"gt{layer}", bufs=2) as gpool,
                      tc.tile_pool(name=f"ad{layer}", bufs=2) as adpool,
                      tc.tile_pool(name=f"ez{layer}", bufs=2) as ezpool,
                      tc.tile_pool(name=f"sS{layer}", bufs=6) as spool,
                      tc.tile_pool(name=f"ev{layer}", bufs=3) as evpool,
                      tc.tile_pool(name=f"pso{layer}", bufs=2, space="PSUM") as pso,
                      tc.tile_pool(name=f"pst{layer}", bufs=2, space="PSUM") as pst,
                      tc.tile_pool(name=f"psf{layer}", bufs=2, space="PSUM") as psf,
                  ):
                      qn = 0
                      for ch in chunks:
                          nblkc = ch["nblk"]
                          ntail = ch["n_tlo"] + ch["n_thi"]
                          gt = gpool.tile([P, CMAXB * HC], FP16, tag="g")
                          gt3 = gt[:].rearrange("p (c e) -> p c e", e=HC)
                          if ch["nlo"]:
                              nc.gpsimd.dma_gather(
                                  gt3[:, 0:ch["nlo"], :], table[0:split, :],
                                  ids_t[:, ch["lo_off"]:
                                        ch["lo_off"] + ch["nlo"] * 8],
                                  ch["nlo"] * P, ch["nlo"] * P, HC,
                                  single_packet=False, queue_num=qn % 4)
                              qn += 1
                          if ch["nhi"]:
                              nc.gpsimd.dma_gather(
                                  gt3[:, ch["nlo"]:nblkc, :],
                                  table[split:n, :],
                                  ids_t[:, ch["hi_off"]:
                                        ch["hi_off"] + ch["nhi"] * 8],
                                  ch["nhi"] * P, ch["nhi"] * P, HC,
                                  single_packet=False, queue_num=qn % 4)
                              qn += 1
                          if ntail:
                              adg = adpool.tile([P, max(TMAXB, 1) * P],
                                                FP16, tag="a")
                              adg3 = adg[:].rearrange("p (c e) -> p c e",
                                                      e=P)
                              if ch["n_tlo"]:
                                  nc.gpsimd.dma_gather(
                                      adg3[:, 0:ch["n_tlo"], :], ad_dram[:],
                                      ids_t[:, ch["adlo_off"]:
                                            ch["adlo_off"]
                                            + ch["n_tlo"] * 8],
                                      ch["n_tlo"] * P, ch["n_tlo"] * P, P,
                                      single_packet=False, queue_num=qn % 4)
                                  qn += 1
                              if ch["n_thi"]:
                                  nc.gpsimd.dma_gather(
                                      adg3[:, ch["n_tlo"]:ntail, :],
                                      ad_dram[:],
                                      ids_t[:, ch["adhi_off"]:
                                            ch["adhi_off"]
                                            + ch["n_thi"] * 8],
                                      ch["n_thi"] * P, ch["n_thi"] * P, P,
                                      single_packet=False, queue_num=qn % 4)
                                  qn += 1

                          if GATHER_ONLY:
                              continue

                          # z = as + ad; ex = exp(lrelu(z)) * mask
                          z4 = ezpool.tile([P, CMAXB * H], FP16, tag="z")
                          z43 = z4[:].rearrange("p (c e) -> p c e", e=H)
                          for g in ch["gs"]:
                              for dkey in ("dl", "dh"):
                                  ds, dn = ch["sec"][(dkey, g)]
                                  if dn == 0:
                                      continue
                                  adv = ad_sb[:, g * H:(g + 1) * H]
                                  adb = bass.AP(adv.tensor, adv.offset,
                                                [adv.ap[0], [0, dn], [1, H]])
                                  nc.vector.tensor_tensor(
                                      out=z43[:, ds:ds + dn, :],
                                      in0=gt3[:, ds:ds + dn, 0:H],
                                      in1=adb, op=mybir.AluOpType.add)
                          if ch["n_tlo"]:
                              nc.vector.tensor_tensor(
                                  out=z43[:, ch["tlo_start"]:
                                          ch["tlo_start"] + ch["n_tlo"], :],
                                  in0=gt3[:, ch["tlo_start"]:
                                          ch["tlo_start"] + ch["n_tlo"],
                                          0:H],
                                  in1=adg3[:, 0:ch["n_tlo"], 0:H],
                                  op=mybir.AluOpType.add)
                          if ch["n_thi"]:
                              nc.vector.tensor_tensor(
                                  out=z43[:, ch["thi_start"]:
                                          ch["thi_start"] + ch["n_thi"], :],
                                  in0=gt3[:, ch["thi_start"]:
                                          ch["thi_start"] + ch["n_thi"],
                                          0:H],
                                  in1=adg3[:, ch["n_tlo"]:ntail, 0:H],
                                  op=mybir.AluOpType.add)
                          lr4 = ezpool.tile([P, CMAXB * H], FP16, tag="l")
                          nc.vector.tensor_scalar(
                              lr4[:, 0:nblkc * H], z4[:, 0:nblkc * H],
                              NEG_SLOPE, None, mybir.AluOpType.mult)
                          nc.vector.tensor_tensor(
                              out=z4[:, 0:nblkc * H],
                              in0=z4[:, 0:nblkc * H],
                              in1=lr4[:, 0:nblkc * H],
                              op=mybir.AluOpType.max)
                          ex4 = ezpool.tile([P, CMAXB * H], FP16, tag="e")
                          nc.scalar.activation(
                              ex4[:, 0:nblkc * H], z4[:, 0:nblkc * H],
                              mybir.ActivationFunctionType.Exp)
                          ex4v = ex4[:].rearrange("p (c e) -> p c e", e=H)
                          mks = mask_t[:, ch["t0"]:ch["t0"] + nblkc]
                          mkb = bass.AP(mks.tensor, mks.offset,
                                        [mks.ap[0], [1, nblkc], [0, H]])
                          nc.vector.tensor_tensor(
                              out=ex4v[:, 0:nblkc, :],
                              in0=ex4v[:, 0:nblkc, :],
                              in1=mkb, op=mybir.AluOpType.mult)
                          # messages *= ex
                          gt4 = gt[:].rearrange("p (c j h) -> p c j h",
                                                j=C, h=H)
                          ex4b = bass.AP(
                              ex4v.tensor, ex4v.offset,
                              [ex4v.ap[0], [H, nblkc], [0, C], [1, H]])
                          nc.vector.tensor_tensor(
                              out=gt4[:, 0:nblkc, :, :],
                              in0=gt4[:, 0:nblkc, :, :],
                              in1=ex4b, op=mybir.AluOpType.mult)

                          for g in ch["gs"]:
                              dls, dln = ch["sec"][("dl", g)]
                              dhs, dhn = ch["sec"][("dh", g)]
                              tls, tln = ch["sec"][("tl", g)]
                              ths, thn = ch["sec"][("th", g)]
                              nb = dln + dhn + tln + thn
                              ps = pso.tile([P, HC], FP32, tag="po")
                              pden = None
                              if tln + thn:
                                  pden = pso.tile([P, H], FP32, tag="pd")
                              j = 0
                              for s0_, n_ in ((dls, dln), (dhs, dhn)):
                                  for jj in range(n_):
                                      nc.tensor.matmul(
                                          ps[:], lhsT=ident_t[:],
                                          rhs=gt3[:, s0_ + jj, :],
                                          start=(j == 0),
                                          stop=(j == nb - 1))
                                      j += 1
                              jt = 0
                              # tail dstloc columns for this group's tail
                              # blocks: global col = tt0 + (sec offset into
                              # this chunk's tail region)
                              for s0_, n_, reg0, regbase in (
                                      (tls, tln, ch["tlo_start"],
                                       ch["tt0"]),
                                      (ths, thn, ch["thi_start"],
                                       ch["tt0"] + ch["n_tlo"])):
                                  for jj in range(n_):
                                      tcolix = regbase + (s0_ - reg0) + jj
                                      S = spool.tile([P, P], FP16, tag="S")
                                      nc.vector.tensor_scalar(
                                          S[:], iota_t[:],
                                          dstloc_t[:, tcolix:tcolix + 1],
                                          None, mybir.AluOpType.is_equal)
                                      nc.tensor.matmul(
                                          ps[:], lhsT=S[:],
                                          rhs=gt3[:, s0_ + jj, :],
                                          start=(j == 0),
                                          stop=(j == nb - 1))
                                      j += 1
                                      nc.tensor.matmul(
                                          pden[:], lhsT=S[:],
                                          rhs=ex4v[:, s0_ + jj, :],
                                          start=(jt == 0),
                                          stop=(jt == tln + thn - 1))
                                      jt += 1

                              # ---- denominators + evacuate ----
                              d1 = evpool.tile([P, H], FP32, tag="d1")
                              dparts = [(s0_, n_) for s0_, n_ in
                                        ((dls, dln), (dhs, dhn)) if n_]
                              if dparts:
                                  s0_, n_ = dparts[0]
                                  dv = bass.AP(ex4v.tensor,
                                               ex4v.offset + s0_ * H,
                                               [ex4v.ap[0], [1, H],
                                                [H, n_]])
                                  nc.vector.tensor_reduce(
                                      out=d1[:], in_=dv,
                                      axis=mybir.AxisListType.X,
                                      op=mybir.AluOpType.add)
                                  if len(dparts) > 1:
                                      s0_, n_ = dparts[1]
                                      d2 = evpool.tile([P, H], FP32,
                                                       tag="d2")
                                      dv2 = bass.AP(ex4v.tensor,
                                                    ex4v.offset + s0_ * H,
                                                    [ex4v.ap[0], [1, H],
                                                     [H, n_]])
                                      nc.vector.tensor_reduce(
                                          out=d2[:], in_=dv2,
                                          axis=mybir.AxisListType.X,
                                          op=mybir.AluOpType.add)
                                      nc.vector.tensor_tensor(
                                          out=d1[:], in0=d1[:], in1=d2[:],
                                          op=mybir.AluOpType.add)
                                  if pden is not None:
                                      nc.vector.tensor_tensor(
                                          out=d1[:], in0=d1[:],
                                          in1=pden[:],
                                          op=mybir.AluOpType.add)
                              else:
                                  nc.vector.tensor_copy(d1[:], pden[:])
                              rec = evpool.tile([P, H], FP32, tag="rec")
                              nc.vector.reciprocal(rec[:], d1[:])
                              recb = bass.AP(
                                  rec[:].tensor, rec[:].offset,
                                  [rec[:].ap[0], [0, C], [1, H]])
                              if first:
                                  nc.vector.tensor_tensor(
                                      out=f1_sb[:, g * HC:(g + 1) * HC],
                                      in0=ps[:], in1=recb,
                                      op=mybir.AluOpType.mult)
                                  if DEBUG_F1 and str(layer) == "0_1":
                                      nc.sync.dma_start(
                                          out=f1out[g * P:(g + 1) * P, :],
                                          in_=f1_sb[:, g * HC:
                                                    (g + 1) * HC])
                              else:
                                  tmp = evpool.tile([P, HC], FP16,
                                                    tag="tmp")
                                  nc.vector.tensor_tensor(
                                      out=tmp[:], in0=ps[:], in1=recb,
                                      op=mybir.AluOpType.mult)
                                  pf = psf.tile([P, out_c], FP32, tag="pf")
                                  for k in range(2):
                                      pt = pst.tile([P, P], FP16, tag="pt")
                                      nc.tensor.transpose(
                                          pt[:], tmp[:, k * P:(k + 1) * P],
                                          ident_t[:])
                                      fT = evpool.tile([P, P], FP16,
                                                       tag="fT")
                                      nc.scalar.copy(fT[:], pt[:])
                                      nc.tensor.matmul(
                                          pf[:], lhsT=fT[:],
                                          rhs=(ma_t if k == 0
                                               else mb_t)[:],
                                          start=(k == 0), stop=(k == 1))
                                  ob = evpool.tile([P, out_c], FP32,
                                                   tag="ob")
                                  nc.vector.tensor_tensor(
                                      out=ob[:], in0=pf[:],
                                      in1=xch_sb[:, g * out_c:
                                                 (g + 1) * out_c],
                                      op=mybir.AluOpType.add)
                                  nc.sync.dma_start(
                                      out=out_d[g * P:(g + 1) * P, :],
                                      in_=ob[:])

              if SKIP_EDGE or GATHER_ONLY:
                  nc.vector.memset(f1_sb[:], 0.0)
                  if GATHER_ONLY:
                      edge_phase(table1, ad1_dram, ad_sb1,
                                 layer=f"{_rep}_1")
              else:
                  edge_phase(table1, ad1_dram, ad_sb1, layer=f"{_rep}_1")

              # ---------------- Phase D ----------------
              with (
                  tc.tile_pool(name=f"pd{_rep}", bufs=3) as pd,
                  tc.tile_pool(name=f"pd_ps{_rep}", bufs=2, space="PSUM") as pd_ps,
                  tc.tile_pool(name=f"pd_pt{_rep}", bufs=2, space="PSUM") as pd_pt,
              ):
                  for g in range(G):
                      ph = pd_ps.tile([P, W2COLS], FP32, tag="ph2")
                      for k in range(2):
                          pft = pd_pt.tile([P, P], FP16, tag="pft")
                          nc.tensor.transpose(
                              pft[:],
                              f1_sb[:, g * HC + k * P:g * HC + (k + 1) * P],
                              ident_t[:])
                          fT = pd.tile([P, P], FP16, tag="fT")
                          nc.scalar.copy(fT[:], pft[:])
                          nc.tensor.matmul(
                              ph[:], lhsT=fT[:],
                              rhs=(w2a_t if k == 0 else w2b_t)[:],
                              start=(k == 0), stop=(k == 1))
                      rows = min(P, npc - g * P)
                      tx = pd.tile([P, HC], FP16, tag="tx2")
                      nc.scalar.copy(tx[:], ph[:, 0:HC])
                      nc.vector.tensor_copy(ad_sb2[:, g * H:(g + 1) * H],
                                            ph[:, HC:W2COLS])
                      nc.sync.dma_start(
                          out=table2_own[g * P:g * P + rows, :],
                          in_=tx[:rows, :])
                      if TB.sum():
                          nc.sync.dma_start(
                              out=ad2_dram[g * P:g * P + rows, 0:H],
                              in_=ad_sb2[:rows, g * H:(g + 1) * H])
                      if DEBUG_F1 and _rep == 0:
                          nc.sync.dma_start(
                              out=t2out[g * P:(g + 1) * P, :], in_=tx[:])
                          nc.sync.dma_start(
                              out=ad2out[g * P:(g + 1) * P, :],
                              in_=ad_sb2[:, g * H:(g + 1) * H])

              if not SKIP_AG:
                  nc.gpsimd.collective_compute(
                      "AllGather",
                      mybir.AluOpType.bypass,
                      replica_groups=[list(range(N_CORES))],
                      ins=[table2_own[:].opt()],
                      outs=[table2[:].opt()],
                  )

              if not (SKIP_EDGE or GATHER_ONLY):
                  edge_phase(table2, ad2_dram, ad_sb2, layer=f"{_rep}_2")
              else:
                  if GATHER_ONLY:
                      edge_phase(table2, ad2_dram, ad_sb2,
                                 layer=f"{_rep}_2")
                  ob0 = cpool.tile([P, out_c], FP32)
                  nc.vector.memset(ob0[:], 0.0)
                  for g in range(G):
                      nc.sync.dma_start(out=out_d[g * P:(g + 1) * P, :],
                                        in_=ob0[:])

    nc.compile()
    return nc


# ---------------------------------------------------------------------------
# Entry point
# ---------------------------------------------------------------------------

def _build_in_maps(x, sched, cores, w1ext, w2d, b1, add_b1):
    G, npc = sched["G"], sched["npc"]
    n, in_c = x.shape
    iota = np.broadcast_to(np.arange(P, dtype=np.float16), (P, P)).copy()
    ident = np.eye(P, dtype=np.float16)
    in_maps = []
    for m in range(N_CORES):
        xpad = np.zeros((G * P, in_c), dtype=np.float32)
        xpad[:npc] = x[m * npc:(m + 1) * npc]
        im = dict(
            xT=np.ascontiguousarray(xpad.T),
            ids16=cores[m]["ids16"],
            dstloc=cores[m]["dstloc"],
            mask=cores[m]["mask"],
            w1ext=w1ext,
            w2ext=w2d["w2ext"],
            mmat=w2d["mmat"],
            iota=iota,
            ident=ident,
        )
        in_maps.append(im)
    return in_maps


def kernel(x, edge_index, W1, a_src1, a_dst1, b1, W2, a_src2, a_dst2, b2,
           Wfc, bfc):
    x = np.asarray(x, dtype=np.float32)
    n, in_c = x.shape
    heads, out_c = np.asarray(a_src1).shape

    sched, cores = _preprocess(edge_index, n, N_CORES)
    npc = sched["npc"]

    w1ext, w2d = _build_weight_ext(W1, a_src1, a_dst1, W2, a_src2, a_dst2,
                                   Wfc, in_c, out_c, heads)
    b1 = np.asarray(b1, np.float64)
    if np.any(b1 != 0):
        raise NotImplementedError("b1 != 0 not supported")
    nc = _build_program(sched, n, in_c, out_c, heads, False)
    in_maps = _build_in_maps(x, sched, cores, w1ext, w2d, b1, False)

    res = run_bass_kernel_spmd(nc, in_maps, list(range(N_CORES)))
    global LAST_RESULTS
    LAST_RESULTS = res
    outs = [res.results[m]["out"][:npc] for m in range(N_CORES)]
    out = np.concatenate(outs, axis=0)
    out = out + (np.asarray(b2) + np.asarray(bfc))[None, :]
    return out.astype(np.float32)


# revision 19
# speedup vs baseline: 1.9355x; 1.2396x over previous
"""Trainium2 Bass kernel for a 2-layer GAT block (gnn_message_passing).

v3: dense dst-major edge blocks.
  - fp16 node tables, rows exactly 512B (256 halves, head-major channel
    layout c*H+h).  Householder rotation trick: alpha_src per edge is
    channels 0..3 of the gathered row; inverse rotations folded into W2 /
    the final mean-over-heads matmul on the host.
  - Edges sorted by dst, sharded by dst range, grouped into 128-dst groups.
    Within each (group, src-half): DENSE dst-major packing -- slot
    (partition p, block j) holds the j-th edge of local dst p (idx-0 pad +
    fp16 mask).  Segment-sum = identity-matmul PSUM accumulation (NO one-hot
    S build), a_dst add is a per-partition SBUF broadcast (NO per-edge ad
    gather), softmax denominators via batched tensor_reduce.  Edges beyond
    the per-(group,half) cap K go to small dst-sorted TAIL blocks using the
    one-hot-S route with a 256B/edge ad gather.
  - Groups processed in chunks of CG; one dma_gather call per (chunk, half)
    plus tiny tail-ad calls.
"""

import numpy as np

import concourse.bass as bass
import concourse.bacc as bacc
import concourse.mybir as mybir
import concourse.tile as tile
from concourse.bass_utils import run_bass_kernel_spmd

N = 50000
E = 800000
IN_C = 128
OUT_C = 64
HEADS = 4
NEG_SLOPE = 0.2
N_CORES = 8

P = 128
CG = 2              # groups per gather chunk

FP32 = mybir.dt.float32
FP16 = mybir.dt.float16
I16 = mybir.dt.int16

# timing-triage flags
SKIP_EDGE = False
SKIP_AG = False
GATHER_ONLY = False
DEBUG_F1 = False


def _ceil_div(a, b):
    return (a + b - 1) // b


def _pack_idxs(flat):
    m = len(flat)
    assert m % 16 == 0
    arr = np.zeros((16, m // 16), np.int16)
    arr[np.arange(m) % 16, np.arange(m) // 16] = flat
    return arr


# ---------------------------------------------------------------------------
# Host-side preprocessing
# ---------------------------------------------------------------------------

def _preprocess(edge_index, n, n_cores):
    npc = n // n_cores
    G = _ceil_div(npc, P)
    split = (n + 1) // 2

    src = np.asarray(edge_index[0], dtype=np.int64)
    dst = np.asarray(edge_index[1], dtype=np.int64)
    loops = np.arange(n, dtype=np.int64)
    src = np.concatenate([src, loops]).astype(np.int32)
    dst = np.concatenate([dst, loops]).astype(np.int32)

    order = np.argsort(dst, kind="stable")
    src = src[order]
    dst = dst[order]
    core_bounds = np.searchsorted(dst, np.arange(0, n + 1, npc))

    percore = []
    deg = np.zeros((n_cores, G, 2, P), np.int64)
    for m in range(n_cores):
        s0, s1 = core_bounds[m], core_bounds[m + 1]
        cs = src[s0:s1]
        cd = dst[s0:s1] - m * npc
        grp = cd >> 7
        hi = (cs >= split).astype(np.int64)
        o = np.lexsort((cs, cd, hi, grp))
        cs, cd, grp, hi = cs[o], cd[o], grp[o], hi[o]
        np.add.at(deg, (m, grp, hi, cd & 127), 1)
        gb = np.searchsorted(grp * 2 + hi, np.arange(2 * G + 2))
        percore.append((cs, cd, gb))

    # per-(group,half) dense cap K and uniform tail block count
    K = np.zeros((G, 2), np.int64)
    TB = np.zeros((G, 2), np.int64)
    for g in range(G):
        for h in range(2):
            d = deg[:, g, h, :]
            dmax = int(d.max())
            best = None
            for k in range(0, dmax + 1):
                tail = int(np.maximum(d - k, 0).sum(axis=1).max())
                tb = _ceil_div(tail, P)
                cost = P * k + 2 * P * tb + 64 * (k + tb)
                if best is None or cost < best[0]:
                    best = (cost, k, tb)
            K[g, h] = best[1]
            TB[g, h] = best[2]

    chunks = []
    t0 = 0
    tt0 = 0
    ids_c = 0
    for c0 in range(0, G, CG):
        gs = list(range(c0, min(c0 + CG, G)))
        sec = {}
        s = 0
        for g in gs:
            sec[("dl", g)] = (s, int(K[g, 0])); s += int(K[g, 0])
        tlo_start = s
        for g in gs:
            sec[("tl", g)] = (s, int(TB[g, 0])); s += int(TB[g, 0])
        nlo = s
        for g in gs:
            sec[("dh", g)] = (s, int(K[g, 1])); s += int(K[g, 1])
        thi_start = s
        for g in gs:
            sec[("th", g)] = (s, int(TB[g, 1])); s += int(TB[g, 1])
        nblk = s
        nhi = nblk - nlo
        n_tlo = sum(int(TB[g, 0]) for g in gs)
        n_thi = sum(int(TB[g, 1]) for g in gs)
        ch = dict(gs=gs, sec=sec, nlo=nlo, nhi=nhi, nblk=nblk, t0=t0,
                  tlo_start=tlo_start, thi_start=thi_start,
                  n_tlo=n_tlo, n_thi=n_thi, tt0=tt0,
                  lo_off=ids_c, hi_off=ids_c + nlo * 8,
                  adlo_off=ids_c + nblk * 8,
                  adhi_off=ids_c + (nblk + n_tlo) * 8)
        chunks.append(ch)
        t0 += nblk
        tt0 += n_tlo + n_thi
        ids_c += (nblk + n_tlo + n_thi) * 8
    B_total = t0
    TB_total = tt0
    ids_cols = ids_c
    CMAXB = max(ch["nblk"] for ch in chunks)
    TMAXB = max((ch["n_tlo"] + ch["n_thi"]) for ch in chunks)

    cores = []
    for m in range(n_cores):
        cs, cd, gb = percore[m]
        mask = np.zeros((P, B_total), np.float16)
        dstloc = np.full((P, max(TB_total, 1)), -1.0, dtype=np.float32)
        ids_parts = []
        tcol = 0
        for ch in chunks:
            idx_slot = np.zeros((P, ch["nblk"]), np.int32)
            ad_lo_l, ad_hi_l = [], []
            for half_pass in (0, 1):
                for g in ch["gs"]:
                    h = half_pass
                    dkey = "dl" if h == 0 else "dh"
                    tkey = "tl" if h == 0 else "th"
                    ad_l = ad_lo_l if h == 0 else ad_hi_l
                    a, b = gb[2 * g + h], gb[2 * g + h + 1]
                    ecs = cs[a:b] - split * h
                    dloc = cd[a:b] & 127
                    k = int(K[g, h])
                    ds, dn = ch["sec"][(dkey, g)]
                    runpos = (np.arange(len(dloc))
                              - np.searchsorted(dloc, dloc, side="left"))
                    dense_sel = runpos < k
                    if k:
                        idx_slot[dloc[dense_sel],
                                 ds + runpos[dense_sel]] = ecs[dense_sel]
                        mk = np.zeros((P, k), np.float16)
                        mk[dloc[dense_sel], runpos[dense_sel]] = 1.0
                        mask[:, ch["t0"] + ds:ch["t0"] + ds + k] = mk
                    ts_, tn = ch["sec"][(tkey, g)]
                    if tn:
                        tsel = ~dense_sel
                        tcs, tcd = ecs[tsel], dloc[tsel]
                        npad = tn * P - len(tcs)
                        assert npad >= 0, (g, h, tn, len(tcs))
                        tflat = np.concatenate(
                            [tcs, np.zeros(npad, np.int32)])
                        tdl = np.concatenate(
                            [tcd.astype(np.float32),
                             np.full(npad, -1.0, np.float32)])
                        tad = np.concatenate(
                            [(tcd + g * P).astype(np.int32),
                             np.zeros(npad, np.int32)])
                        idx_slot[:, ts_:ts_ + tn] = tflat.reshape(tn, P).T
                        mask[:, ch["t0"] + ts_:ch["t0"] + ts_ + tn] = 1.0
                        dstloc[:, tcol:tcol + tn] = tdl.reshape(tn, P).T
                        tcol += tn
                        ad_l.append(tad)
                    else:
                        assert (~dense_sel).sum() == 0
            # phantom dsts (zero edges in this group, e.g. node ids
            # beyond npc in the last group): force one masked-in pad slot
            # so their softmax denominator is nonzero (avoids 0*inf NaNs
            # that the identity-matmul would spread across partitions).
            for g in ch["gs"]:
                dtot = deg[m, g].sum(axis=0)          # [P]
                ph_d = np.where(dtot == 0)[0]
                if len(ph_d):
                    for dkey in ("dl", "dh"):
                        ds, dn = ch["sec"][(dkey, g)]
                        if dn:
                            mask[ph_d, ch["t0"] + ds] = 1.0
                            break
                    else:
                        raise AssertionError("group with no dense slots")
            lo_flat = idx_slot[:, 0:ch["nlo"]].T.ravel()
            hi_flat = idx_slot[:, ch["nlo"]:ch["nblk"]].T.ravel()
            ids_parts.append(_pack_idxs(lo_flat.astype(np.int16)))
            ids_parts.append(_pack_idxs(hi_flat.astype(np.int16)))
            ad_flat = (np.concatenate(ad_lo_l + ad_hi_l)
                       if (ad_lo_l or ad_hi_l) else np.zeros(0, np.int32))
            ids_parts.append(_pack_idxs(ad_flat.astype(np.int16)))
        assert tcol == TB_total
        ids16 = np.tile(np.concatenate(ids_parts, axis=1), (8, 1))
        assert ids16.shape == (P, ids_cols), (ids16.shape, ids_cols)
        cores.append(dict(ids16=ids16, dstloc=dstloc, mask=mask))

    sched = dict(G=G, npc=npc, split=split, K=K, TB=TB,
                 B_total=B_total, TB_total=TB_total, CMAXB=CMAXB,
                 TMAXB=TMAXB, ids_cols=ids_cols, chunks=chunks)
    return sched, cores


# ---------------------------------------------------------------------------
# Weight preparation (rotation trick, head-major layout)
# ---------------------------------------------------------------------------

def _scaled_rot(a):
    a = np.asarray(a, np.float64)
    nrm = np.linalg.norm(a)
    C = len(a)
    e1 = np.zeros(C); e1[0] = 1.0
    v = a / nrm - e1
    if np.linalg.norm(v) < 1e-12:
        Q = np.eye(C)
    else:
        v /= np.linalg.norm(v)
        Q = np.eye(C) - 2.0 * np.outer(v, v)
    R = Q.copy()
    R[0, :] = a
    return R


def _build_weight_ext(W1, a_src1, a_dst1, W2, a_src2, a_dst2, Wfc,
                      in_c, out_c, heads):
    H, C = heads, out_c
    HC = H * C
    f = lambda x: np.asarray(x, np.float64)
    W1, a_src1, a_dst1 = f(W1), f(a_src1), f(a_dst1)
    W2, a_src2, a_dst2 = f(W2), f(a_src2), f(a_dst2)
    Wfc = f(Wfc)

    PHM = np.zeros(HC, np.int64)
    for h in range(H):
        for c in range(C):
            PHM[c * H + h] = h * C + c

    R1 = [_scaled_rot(a_src1[h]) for h in range(H)]
    R2 = [_scaled_rot(a_src2[h]) for h in range(H)]

    W1r = W1.reshape(in_c, H, C)
    W1rot = np.concatenate([W1r[:, h, :] @ R1[h].T for h in range(H)], axis=1)
    ad1cols = np.einsum("khc,hc->kh", W1r, a_dst1)
    w1ext = np.concatenate([W1rot[:, PHM], ad1cols, Wfc], axis=1)

    B = np.zeros((HC, HC))
    for h in range(H):
        B[h * C:(h + 1) * C, :] = \
            np.linalg.inv(R1[h]).T @ W2[h * C:(h + 1) * C, :]
    C2 = np.concatenate(
        [B[:, h * C:(h + 1) * C] @ R2[h].T for h in range(H)], axis=1)
    ad2_mat = np.zeros((HC, H))
    for h in range(H):
        ad2_mat[:, h] = B[:, h * C:(h + 1) * C] @ a_dst2[h]
    w2full = np.concatenate([C2[:, PHM], ad2_mat], axis=1)
    w2ext = w2full[PHM, :]

    M_std = np.zeros((HC, C))
    for h in range(H):
        M_std[h * C:(h + 1) * C, :] = np.linalg.inv(R2[h]).T / H
    M_hm = M_std[PHM, :]

    return w1ext.astype(np.float32), dict(
        w2ext=w2ext.astype(np.float16), mmat=M_hm.astype(np.float16))


# ---------------------------------------------------------------------------
# Device program
# ---------------------------------------------------------------------------

def _build_program(sched, n, in_c, out_c, heads, add_b1, reps=1):
    G = sched["G"]
    npc = sched["npc"]
    split = sched["split"]
    K, TB = sched["K"], sched["TB"]
    B_total = sched["B_total"]
    TB_total = sched["TB_total"]
    CMAXB = sched["CMAXB"]
    TMAXB = sched["TMAXB"]
    ids_cols = sched["ids_cols"]
    chunks = sched["chunks"]
    H, C = heads, out_c
    HC = H * C
    W1COLS = HC + H + out_c
    W2COLS = HC + H

    nc = bacc.Bacc(
        "TRN2",
        target_bir_lowering=False,
        debug=False,
        enable_asserts=False,
        num_devices=N_CORES,
        num_swdge_queues=4,
    )

    xT = nc.dram_tensor("xT", [in_c, G * P], FP32, kind="ExternalInput")
    ids_d = nc.dram_tensor("ids16", [P, ids_cols], I16, kind="ExternalInput")
    dstloc_d = nc.dram_tensor("dstloc", [P, max(TB_total, 1)], FP32,
                              kind="ExternalInput")
    mask_d = nc.dram_tensor("mask", [P, B_total], FP16, kind="ExternalInput")
    w1ext_d = nc.dram_tensor("w1ext", [in_c, W1COLS], FP32,
                             kind="ExternalInput")
    w2ext_d = nc.dram_tensor("w2ext", [HC, W2COLS], FP16,
                             kind="ExternalInput")
    mmat_d = nc.dram_tensor("mmat", [HC, out_c], FP16, kind="ExternalInput")
    iota_d = nc.dram_tensor("iota", [P, P], FP16, kind="ExternalInput")
    ident_d = nc.dram_tensor("ident", [P, P], FP16, kind="ExternalInput")
    out_d = nc.dram_tensor("out", [G * P, out_c], FP32, kind="ExternalOutput")
    if DEBUG_F1:
        f1out = nc.dram_tensor("f1out", [G * P, HC], FP16,
                               kind="ExternalOutput")
        t1out = nc.dram_tensor("t1out", [G * P, HC], FP16,
                               kind="ExternalOutput")
        ad1out = nc.dram_tensor("ad1out", [G * P, H], FP16,
                                kind="ExternalOutput")
        t2out = nc.dram_tensor("t2out", [G * P, HC], FP16,
                               kind="ExternalOutput")
        ad2out = nc.dram_tensor("ad2out", [G * P, H], FP16,
                                kind="ExternalOutput")

    with tile.TileContext(nc) as tc:
        with (
            tc.tile_pool(name="const", bufs=1) as cpool,
            tc.tile_pool(name="dram", bufs=1, space="DRAM") as dpool,
        ):
            iota_t = cpool.tile([P, P], FP16)
            nc.sync.dma_start(out=iota_t[:], in_=iota_d[:])
            ident_t = cpool.tile([P, P], FP16)
            nc.sync.dma_start(out=ident_t[:], in_=ident_d[:])
            w1_t = cpool.tile([in_c, W1COLS], FP32)
            nc.sync.dma_start(out=w1_t[:], in_=w1ext_d[:])
            w2a_t = cpool.tile([P, W2COLS], FP16)
            nc.sync.dma_start(out=w2a_t[:], in_=w2ext_d[0:P, :])
            w2b_t = cpool.tile([P, W2COLS], FP16)
            nc.sync.dma_start(out=w2b_t[:], in_=w2ext_d[P:2 * P, :])
            ma_t = cpool.tile([P, out_c], FP16)
            nc.sync.dma_start(out=ma_t[:], in_=mmat_d[0:P, :])
            mb_t = cpool.tile([P, out_c], FP16)
            nc.sync.dma_start(out=mb_t[:], in_=mmat_d[P:2 * P, :])
            ids_t = cpool.tile([P, ids_cols], I16)
            nc.sync.dma_start(out=ids_t[:], in_=ids_d[:])
            dstloc_t = cpool.tile([P, max(TB_total, 1)], FP32)
            nc.sync.dma_start(out=dstloc_t[:], in_=dstloc_d[:])
            mask_t = cpool.tile([P, B_total], FP16)
            nc.sync.dma_start(out=mask_t[:], in_=mask_d[:])

            f1_sb = cpool.tile([P, G * HC], FP16)
            xch_sb = cpool.tile([P, G * out_c], FP32)
            ad_sb1 = cpool.tile([P, G * H], FP16)
            ad_sb2 = cpool.tile([P, G * H], FP16)

            for _rep in range(reps):
              table1_own = dpool.tile([npc, HC], FP16, tag=f"t1o{_rep}",
                                      name=f"table1_own{_rep}")
              table1 = dpool.tile([n, HC], FP16, addr_space="Shared",
                                  tag=f"t1{_rep}", name=f"table1{_rep}")
              table2_own = dpool.tile([npc, HC], FP16, tag=f"t2o{_rep}",
                                      name=f"table2_own{_rep}")
              table2 = dpool.tile([n, HC], FP16, addr_space="Shared",
                                  tag=f"t2{_rep}", name=f"table2{_rep}")
              ad1_dram = dpool.tile([npc, P], FP16, tag=f"a1{_rep}",
                                    name=f"ad1_dram{_rep}")
              ad2_dram = dpool.tile([npc, P], FP16, tag=f"a2{_rep}",
                                    name=f"ad2_dram{_rep}")

              # ---------------- Phase A ----------------
              with (
                  tc.tile_pool(name=f"pa{_rep}", bufs=3) as pa,
                  tc.tile_pool(name=f"pa_ps{_rep}", bufs=2, space="PSUM") as pa_ps,
              ):
                  for g in range(G):
                      xt_t = pa.tile([in_c, P], FP32, tag="xt")
                      nc.sync.dma_start(out=xt_t[:],
                                        in_=xT[:, g * P:(g + 1) * P])
                      ph = pa_ps.tile([P, W1COLS], FP32, tag="ph")
                      nc.tensor.matmul(ph[:], lhsT=xt_t[:], rhs=w1_t[:],
                                       start=True, stop=True)
                      rows = min(P, npc - g * P)
                      tx = pa.tile([P, HC], FP16, tag="tx")
                      nc.scalar.copy(tx[:], ph[:, 0:HC])
                      nc.vector.tensor_copy(ad_sb1[:, g * H:(g + 1) * H],
                                            ph[:, HC:HC + H])
                      nc.vector.tensor_copy(
                          xch_sb[:, g * out_c:(g + 1) * out_c],
                          ph[:, HC + H:W1COLS])
                      nc.sync.dma_start(
                          out=table1_own[g * P:g * P + rows, :],
                          in_=tx[:rows, :])
                      if TB.sum():
                          nc.sync.dma_start(
                              out=ad1_dram[g * P:g * P + rows, 0:H],
                              in_=ad_sb1[:rows, g * H:(g + 1) * H])
                      if DEBUG_F1 and _rep == 0:
                          nc.sync.dma_start(
                              out=t1out[g * P:(g + 1) * P, :], in_=tx[:])
                          nc.sync.dma_start(
                              out=ad1out[g * P:(g + 1) * P, :],
                              in_=ad_sb1[:, g * H:(g + 1) * H])

              if not SKIP_AG:
                  nc.gpsimd.collective_compute(
                      "AllGather",
                      mybir.AluOpType.bypass,
                      replica_groups=[list(range(N_CORES))],
                      ins=[table1_own[:].opt()],
                      outs=[table1[:].opt()],
                  )

              # ---------------- Edge phase ----------------
              def edge_phase(table, ad_dram, ad_sb, layer):
                  first = str(layer).endswith("_1")
                  with (
                      tc.tile_pool(name=f"gt{layer}", bufs=3) as gpool,
                      tc.tile_pool(name=f"ad{layer}", bufs=2) as adpool,
                      tc.tile_pool(name=f"ez{layer}", bufs=3) as ezpool,
                      tc.tile_pool(name=f"sS{layer}", bufs=10) as spool,
                      tc.tile_pool(name=f"ev{layer}", bufs=4) as evpool,
                      tc.tile_pool(name=f"pso{layer}", bufs=(3 if first else 2),
                                   space="PSUM") as pso,
                      tc.tile_pool(name=f"pst{layer}", bufs=2, space="PSUM") as pst,
                      tc.tile_pool(name=f"psf{layer}", bufs=2, space="PSUM") as psf,
                  ):
                      qn = 0
                      for ch in chunks:
                          nblkc = ch["nblk"]
                          ntail = ch["n_tlo"] + ch["n_thi"]
                          gt = gpool.tile([P, CMAXB * HC], FP16, tag="g")
                          gt3 = gt[:].rearrange("p (c e) -> p c e", e=HC)
                          if ch["nlo"]:
                              nc.gpsimd.dma_gather(
                                  gt3[:, 0:ch["nlo"], :], table[0:split, :],
                                  ids_t[:, ch["lo_off"]:
                                        ch["lo_off"] + ch["nlo"] * 8],
                                  ch["nlo"] * P, ch["nlo"] * P, HC,
                                  single_packet=False, queue_num=qn % 4)
                              qn += 1
                          if ch["nhi"]:
                              nc.gpsimd.dma_gather(
                                  gt3[:, ch["nlo"]:nblkc, :],
                                  table[split:n, :],
                                  ids_t[:, ch["hi_off"]:
                                        ch["hi_off"] + ch["nhi"] * 8],
                                  ch["nhi"] * P, ch["nhi"] * P, HC,
                                  single_packet=False, queue_num=qn % 4)
                              qn += 1
                          if ntail:
                              adg = adpool.tile([P, max(TMAXB, 1) * P],
                                                FP16, tag="a")
                              adg3 = adg[:].rearrange("p (c e) -> p c e",
                                                      e=P)
                              if ch["n_tlo"]:
                                  nc.gpsimd.dma_gather(
                                      adg3[:, 0:ch["n_tlo"], :], ad_dram[:],
                                      ids_t[:, ch["adlo_off"]:
                                            ch["adlo_off"]
                                            + ch["n_tlo"] * 8],
                                      ch["n_tlo"] * P, ch["n_tlo"] * P, P,
                                      single_packet=False, queue_num=qn % 4)
                                  qn += 1
                              if ch["n_thi"]:
                                  nc.gpsimd.dma_gather(
                                      adg3[:, ch["n_tlo"]:ntail, :],
                                      ad_dram[:],
                                      ids_t[:, ch["adhi_off"]:
                                            ch["adhi_off"]
                                            + ch["n_thi"] * 8],
                                      ch["n_thi"] * P, ch["n_thi"] * P, P,
                                      single_packet=False, queue_num=qn % 4)
                                  qn += 1

                          if GATHER_ONLY:
                              continue

                          # z = as + ad; ex = exp(lrelu(z)) * mask
                          z4 = ezpool.tile([P, CMAXB * H], FP16, tag="z")
                          z43 = z4[:].rearrange("p (c e) -> p c e", e=H)
                          for g in ch["gs"]:
                              for dkey in ("dl", "dh"):
                                  ds, dn = ch["sec"][(dkey, g)]
                                  if dn == 0:
                                      continue
                                  adv = ad_sb[:, g * H:(g + 1) * H]
                                  adb = bass.AP(adv.tensor, adv.offset,
                                                [adv.ap[0], [0, dn], [1, H]])
                                  nc.vector.tensor_tensor(
                                      out=z43[:, ds:ds + dn, :],
                                      in0=gt3[:, ds:ds + dn, 0:H],
                                      in1=adb, op=mybir.AluOpType.add)
                          if ch["n_tlo"]:
                              nc.vector.tensor_tensor(
                                  out=z43[:, ch["tlo_start"]:
                                          ch["tlo_start"] + ch["n_tlo"], :],
                                  in0=gt3[:, ch["tlo_start"]:
                                          ch["tlo_start"] + ch["n_tlo"],
                                          0:H],
                                  in1=adg3[:, 0:ch["n_tlo"], 0:H],
                                  op=mybir.AluOpType.add)
                          if ch["n_thi"]:
                              nc.vector.tensor_tensor(
                                  out=z43[:, ch["thi_start"]:
                                          ch["thi_start"] + ch["n_thi"], :],
                                  in0=gt3[:, ch["thi_start"]:
                                          ch["thi_start"] + ch["n_thi"],
                                          0:H],
                                  in1=adg3[:, ch["n_tlo"]:ntail, 0:H],
                                  op=mybir.AluOpType.add)
                          lr4 = ezpool.tile([P, CMAXB * H], FP16, tag="l")
                          nc.vector.tensor_scalar(
                              lr4[:, 0:nblkc * H], z4[:, 0:nblkc * H],
                              NEG_SLOPE, None, mybir.AluOpType.mult)
                          nc.vector.tensor_tensor(
                              out=z4[:, 0:nblkc * H],
                              in0=z4[:, 0:nblkc * H],
                              in1=lr4[:, 0:nblkc * H],
                              op=mybir.AluOpType.max)
                          ex4 = ezpool.tile([P, CMAXB * H], FP16, tag="e")
                          nc.scalar.activation(
                              ex4[:, 0:nblkc * H], z4[:, 0:nblkc * H],
                              mybir.ActivationFunctionType.Exp)
                          ex4v = ex4[:].rearrange("p (c e) -> p c e", e=H)
                          mks = mask_t[:, ch["t0"]:ch["t0"] + nblkc]
                          mkb = bass.AP(mks.tensor, mks.offset,
                                        [mks.ap[0], [1, nblkc], [0, H]])
                          nc.vector.tensor_tensor(
                              out=ex4v[:, 0:nblkc, :],
                              in0=ex4v[:, 0:nblkc, :],
                              in1=mkb, op=mybir.AluOpType.mult)
                          # messages *= ex
                          gt4 = gt[:].rearrange("p (c j h) -> p c j h",
                                                j=C, h=H)
                          ex4b = bass.AP(
                              ex4v.tensor, ex4v.offset,
                              [ex4v.ap[0], [H, nblkc], [0, C], [1, H]])
                          nc.vector.tensor_tensor(
                              out=gt4[:, 0:nblkc, :, :],
                              in0=gt4[:, 0:nblkc, :, :],
                              in1=ex4b, op=mybir.AluOpType.mult)

                          for g in ch["gs"]:
                              dls, dln = ch["sec"][("dl", g)]
                              dhs, dhn = ch["sec"][("dh", g)]
                              tls, tln = ch["sec"][("tl", g)]
                              ths, thn = ch["sec"][("th", g)]
                              nb = dln + dhn + tln + thn
                              ps = pso.tile([P, HC], FP32, tag="po")
                              pden = None
                              if tln + thn:
                                  pden = pso.tile([P, H], FP32, tag="pd")
                              j = 0
                              for s0_, n_ in ((dls, dln), (dhs, dhn)):
                                  for jj in range(n_):
                                      nc.tensor.matmul(
                                          ps[:], lhsT=ident_t[:],
                                          rhs=gt3[:, s0_ + jj, :],
                                          start=(j == 0),
                                          stop=(j == nb - 1))
                                      j += 1
                              jt = 0
                              # tail dstloc columns for this group's tail
                              # blocks: global col = tt0 + (sec offset into
                              # this chunk's tail region)
                              for s0_, n_, reg0, regbase in (
                                      (tls, tln, ch["tlo_start"],
                                       ch["tt0"]),
                                      (ths, thn, ch["thi_start"],
                                       ch["tt0"] + ch["n_tlo"])):
                                  for jj in range(n_):
                                      tcolix = regbase + (s0_ - reg0) + jj
                                      S = spool.tile([P, P], FP16, tag="S")
                                      nc.vector.tensor_scalar(
                                          S[:], iota_t[:],
                                          dstloc_t[:, tcolix:tcolix + 1],
                                          None, mybir.AluOpType.is_equal)
                                      nc.tensor.matmul(
                                          ps[:], lhsT=S[:],
                                          rhs=gt3[:, s0_ + jj, :],
                                          start=(j == 0),
                                          stop=(j == nb - 1))
                                      j += 1
                                      nc.tensor.matmul(
                                          pden[:], lhsT=S[:],
                                          rhs=ex4v[:, s0_ + jj, :],
                                          start=(jt == 0),
                                          stop=(jt == tln + thn - 1))
                                      jt += 1

                              # ---- denominators + evacuate ----
                              d1 = evpool.tile([P, H], FP32, tag="d1")
                              dparts = [(s0_, n_) for s0_, n_ in
                                        ((dls, dln), (dhs, dhn)) if n_]
                              if dparts:
                                  s0_, n_ = dparts[0]
                                  dv = bass.AP(ex4v.tensor,
                                               ex4v.offset + s0_ * H,
                                               [ex4v.ap[0], [1, H],
                                                [H, n_]])
                                  nc.vector.tensor_reduce(
                                      out=d1[:], in_=dv,
                                      axis=mybir.AxisListType.X,
                                      op=mybir.AluOpType.add)
                                  if len(dparts) > 1:
                                      s0_, n_ = dparts[1]
                                      d2 = evpool.tile([P, H], FP32,
                                                       tag="d2")
                                      dv2 = bass.AP(ex4v.tensor,
                                                    ex4v.offset + s0_ * H,
                                                    [ex4v.ap[0], [1, H],
                                                     [H, n_]])
                                      nc.vector.tensor_reduce(
                                          out=d2[:], in_=dv2,
                                          axis=mybir.AxisListType.X,
                                          op=mybir.AluOpType.add)
                                      nc.vector.tensor_tensor(
                                          out=d1[:], in0=d1[:], in1=d2[:],
                                          op=mybir.AluOpType.add)
                                  if pden is not None:
                                      nc.vector.tensor_tensor(
                                          out=d1[:], in0=d1[:],
                                          in1=pden[:],
                                          op=mybir.AluOpType.add)
                              else:
                                  nc.vector.tensor_copy(d1[:], pden[:])
                              rec = evpool.tile([P, H], FP32, tag="rec")
                              nc.vector.reciprocal(rec[:], d1[:])
                              recb = bass.AP(
                                  rec[:].tensor, rec[:].offset,
                                  [rec[:].ap[0], [0, C], [1, H]])
                              if first:
                                  nc.vector.tensor_tensor(
                                      out=f1_sb[:, g * HC:(g + 1) * HC],
                                      in0=ps[:], in1=recb,
                                      op=mybir.AluOpType.mult)
                                  if DEBUG_F1 and str(layer) == "0_1":
                                      nc.sync.dma_start(
                                          out=f1out[g * P:(g + 1) * P, :],
                                          in_=f1_sb[:, g * HC:
                                                    (g + 1) * HC])
                              else:
                                  tmp = evpool.tile([P, HC], FP16,
                                                    tag="tmp")
                                  nc.vector.tensor_tensor(
                                      out=tmp[:], in0=ps[:], in1=recb,
                                      op=mybir.AluOpType.mult)
                                  pf = psf.tile([P, out_c], FP32, tag="pf")
                                  for k in range(2):
                                      pt = pst.tile([P, P], FP16, tag="pt")
                                      nc.tensor.transpose(
                                          pt[:], tmp[:, k * P:(k + 1) * P],
                                          ident_t[:])
                                      fT = evpool.tile([P, P], FP16,
                                                       tag="fT")
                                      nc.scalar.copy(fT[:], pt[:])
                                      nc.tensor.matmul(
                                          pf[:], lhsT=fT[:],
                                          rhs=(ma_t if k == 0
                                               else mb_t)[:],
                                          start=(k == 0), stop=(k == 1))
                                  ob = evpool.tile([P, out_c], FP32,
                                                   tag="ob")
                                  nc.vector.tensor_tensor(
                                      out=ob[:], in0=pf[:],
                                      in1=xch_sb[:, g * out_c:
                                                 (g + 1) * out_c],
                                      op=mybir.AluOpType.add)
                                  nc.sync.dma_start(
                                      out=out_d[g * P:(g + 1) * P, :],
                                      in_=ob[:])

              if SKIP_EDGE or GATHER_ONLY:
                  nc.vector.memset(f1_sb[:], 0.0)
                  if GATHER_ONLY:
                      edge_phase(table1, ad1_dram, ad_sb1,
                                 layer=f"{_rep}_1")
              else:
                  edge_phase(table1, ad1_dram, ad_sb1, layer=f"{_rep}_1")

              # ---------------- Phase D ----------------
              with (
                  tc.tile_pool(name=f"pd{_rep}", bufs=3) as pd,
                  tc.tile_pool(name=f"pd_ps{_rep}", bufs=2, space="PSUM") as pd_ps,
                  tc.tile_pool(name=f"pd_pt{_rep}", bufs=2, space="PSUM") as pd_pt,
              ):
                  for g in range(G):
                      ph = pd_ps.tile([P, W2COLS], FP32, tag="ph2")
                      for k in range(2):
                          pft = pd_pt.tile([P, P], FP16, tag="pft")
                          nc.tensor.transpose(
                              pft[:],
                              f1_sb[:, g * HC + k * P:g * HC + (k + 1) * P],
                              ident_t[:])
                          fT = pd.tile([P, P], FP16, tag="fT")
                          nc.scalar.copy(fT[:], pft[:])
                          nc.tensor.matmul(
                              ph[:], lhsT=fT[:],
                              rhs=(w2a_t if k == 0 else w2b_t)[:],
                              start=(k == 0), stop=(k == 1))
                      rows = min(P, npc - g * P)
                      tx = pd.tile([P, HC], FP16, tag="tx2")
                      nc.scalar.copy(tx[:], ph[:, 0:HC])
                      nc.vector.tensor_copy(ad_sb2[:, g * H:(g + 1) * H],
                                            ph[:, HC:W2COLS])
                      nc.sync.dma_start(
                          out=table2_own[g * P:g * P + rows, :],
                          in_=tx[:rows, :])
                      if TB.sum():
                          nc.sync.dma_start(
                              out=ad2_dram[g * P:g * P + rows, 0:H],
                              in_=ad_sb2[:rows, g * H:(g + 1) * H])
                      if DEBUG_F1 and _rep == 0:
                          nc.sync.dma_start(
                              out=t2out[g * P:(g + 1) * P, :], in_=tx[:])
                          nc.sync.dma_start(
                              out=ad2out[g * P:(g + 1) * P, :],
                              in_=ad_sb2[:, g * H:(g + 1) * H])

              if not SKIP_AG:
                  nc.gpsimd.collective_compute(
                      "AllGather",
                      mybir.AluOpType.bypass,
                      replica_groups=[list(range(N_CORES))],
                      ins=[table2_own[:].opt()],
                      outs=[table2[:].opt()],
                  )

              if not (SKIP_EDGE or GATHER_ONLY):
                  edge_phase(table2, ad2_dram, ad_sb2, layer=f"{_rep}_2")
              else:
                  if GATHER_ONLY:
                      edge_phase(table2, ad2_dram, ad_sb2,
                                 layer=f"{_rep}_2")
                  ob0 = cpool.tile([P, out_c], FP32)
                  nc.vector.memset(ob0[:], 0.0)
                  for g in range(G):
                      nc.sync.dma_start(out=out_d[g * P:(g + 1) * P, :],
                                        in_=ob0[:])

    nc.compile()
    return nc


# ---------------------------------------------------------------------------
# Entry point
# ---------------------------------------------------------------------------

def _build_in_maps(x, sched, cores, w1ext, w2d, b1, add_b1):
    G, npc = sched["G"], sched["npc"]
    n, in_c = x.shape
    iota = np.broadcast_to(np.arange(P, dtype=np.float16), (P, P)).copy()
    ident = np.eye(P, dtype=np.float16)
    in_maps = []
    for m in range(N_CORES):
        xpad = np.zeros((G * P, in_c), dtype=np.float32)
        xpad[:npc] = x[m * npc:(m + 1) * npc]
        im = dict(
            xT=np.ascontiguousarray(xpad.T),
            ids16=cores[m]["ids16"],
            dstloc=cores[m]["dstloc"],
            mask=cores[m]["mask"],
            w1ext=w1ext,
            w2ext=w2d["w2ext"],
            mmat=w2d["mmat"],
            iota=iota,
            ident=ident,
        )
        in_maps.append(im)
    return in_maps


def kernel(x, edge_index, W1, a_src1, a_dst1, b1, W2, a_src2, a_dst2, b2,
           Wfc, bfc):
    x = np.asarray(x, dtype=np.float32)
    n, in_c = x.shape
    heads, out_c = np.asarray(a_src1).shape

    sched, cores = _preprocess(edge_index, n, N_CORES)
    npc = sched["npc"]

    w1ext, w2d = _build_weight_ext(W1, a_src1, a_dst1, W2, a_src2, a_dst2,
                                   Wfc, in_c, out_c, heads)
    b1 = np.asarray(b1, np.float64)
    if np.any(b1 != 0):
        raise NotImplementedError("b1 != 0 not supported")
    nc = _build_program(sched, n, in_c, out_c, heads, False)
    in_maps = _build_in_maps(x, sched, cores, w1ext, w2d, b1, False)

    res = run_bass_kernel_spmd(nc, in_maps, list(range(N_CORES)))
    global LAST_RESULTS
    LAST_RESULTS = res
    outs = [res.results[m]["out"][:npc] for m in range(N_CORES)]
    out = np.concatenate(outs, axis=0)
    out = out + (np.asarray(b2) + np.asarray(bfc))[None, :]
    return out.astype(np.float32)
